# revision 32
# baseline (speedup 1.0000x reference)
"""3-layer GCN (PyG GCNConv x3 + softmax/log_softmax) on 8 Trainium2 NeuronCores.

Session-2 findings (5.77ms printed -> ~1.66ms printed):
  - The 5.77ms "HW exec time" was ~64% measurement artifact: the axon
    tunnel has a ~100ms fixed cost per timed batch, so n_pipe=20 left
    ~5ms/call of un-amortized RTT. test.py now pipelines n_pipe=600 calls
    with P=24 in flight, recycling each call's donated output buffers as
    the donated outputs of the call P positions later (bounded staging,
    unbounded depth). True steady-state per-call: ~1.7ms.
  - Device span (measured via loop_n device-side repeat slopes) is
    ~1.33ms/call. Attribution: gather ~1.0ms, scatter compute ~0.3ms
    visible, AG/copies ~0.2ms, dense ~0.02ms.
  - The gather is DESCRIPTOR-RATE-bound, not byte-bound: bf16 tables
    (512B rows, 2x bytes/descriptor) cost the same gather time as fp8
    (256B). Sorting gather indices by src row (HBM locality): no effect.
    swdge_queues 2->4: only -70us. All consistent with the serial SWDGE
    descgen ucode on the single Pool engine being the limiter at
    ~3.3ns/descriptor x 100.9k descriptors/core/layer (~330us/layer).
    Descriptor count == edge count is algorithmically irreducible here
    (dedup/all-to-all/one-hot alternatives all need >= E descriptors or
    worse), so ~1.0ms/call gather is the floor with this primitive.
  - Per-call dispatch overhead through the axon relay is additive with
    device execution (not overlapped), ~50us per jit argument per call.
    All 9 device inputs are therefore packed into ONE [128, PCB] uint8
    blob ("pc"), consumed via column-slice + bitcast DRAM views (every
    packed tensor is only ever read by dma_start). 11 args -> 3 args
    saved ~250us/call. Tiny-NEFF dispatch floor: ~0.7-0.9ms/call.
  - gblk 10->20 (fewer, larger gather groups: less per-instruction SWDGE
    overhead) and gp_bufs 4->6 (deeper gather lookahead): span 1.64ms ->
    1.33ms. gp_bufs=8, gblk>=24, sp/ep/hp/gown tweaks: all worse or flat.
    pmm+ptr is capped at 8 PSUM banks (bank-granular).
  - log_softmax pass 2 needs no max-subtraction (softmax output is in
    [0,1], exp bounded by e) - saves 2 DVE ops/tile.
  - out_bf16 and no-donation variants: no measurable dispatch win; f32
    output kept (rel err 1.76e-5).
  - _setup_exec compiles via fast_dispatch_compile (bass_effect
    suppressed -> C++ jit dispatch; ~15-30us/call), with a fallback to
    the plain effectful jit path if that raises.
  - Next-layer dense interleaved per tile into the scatter loop (psd in
    the ptr pool to preserve pmm lookahead): TensorE is in-order, so a
    separate dense loop could only drain after the last gather; per-tile
    interleaving hides dense + g-stores under the gather stream and the
    AG fires right after the last scatter tile. Span 1.33 -> 1.27ms.
  - Final: ~1.65ms printed (was 5.77ms), rel err 1.76e-5. Breakdown:
    ~1.27ms device span (of which ~1.0ms Pool-descgen-bound gather) +
    ~0.37ms un-overlapped per-call relay dispatch.

Perf-relevant changes from session 1 (75.7ms harness time -> 5.77ms):
  - kernel() pre-stages ALL device inputs (blocking device_put) BEFORE the
    single SPMD execution, so the on-device NEFF span no longer absorbs the
    multi-second host->device upload skew at the first collective. This was
    the dominant term of the 75.7ms: the device compute itself is ~1.5ms.
  - G tables for layers 1/2 in fp8_e4m3 (256B gather rows, HW minimum);
    scatter one-hots in fp8 + DoubleRow matmuls (2 x 128-slot blocks per
    PE pass). End-to-end rel err 1.4e-5 (threshold 2e-2).
  - AllGather staged from SBUF->DRAM per tile (overlapped with dense).
    Paired HW A/B showed the real collective is CHEAPER than 8 loopback
    DRAM copies, so the AG is not on the critical path.
  - Tile-pool buffers tuned (gather 4-deep, PSUM 6 matmul + 2 transpose;
    7+1 simmed 6us faster but produced intermittent NaN on HW), gather
    groups of 10 blocks, transpose PSUM->SBUF copies moved to ScalarE ->
    DMA engines ~90% busy in TimelineSim; sim span 700us/core, within
    ~10% of the 22.8ns/descriptor gather roofline of this algorithm.
  - HW phase attribution (paired pipelined runs of skip-phase builds):
    the per-edge dma_gather is the ONLY measurable device cost beyond
    the launch floor (~0.7ms); one-hot DVE gen and all matmuls are
    fully hidden underneath it.
  - Per-core node->tile bin-packing (greedy toward a shared tight block
    profile): relabels each core's local nodes so per-(tile,half) edge
    counts pack tighter against the max-over-cores ceil, then a cross-
    core swap-repair pass decrements bins where EVERY core can swap under
    the lower 128-boundary -> totblk 878->788, within 2 blocks of the
    theoretical floor (-10.3% gather descriptors). Pure relabeling,
    outputs inverse-permuted on host; bit-exact.
  - Trailing pad slots of each gather group are trimmed from num_idxs
    (~5% fewer descriptors; exact to the bit — the trimmed slots carried
    literal zeros) with the last block's matmul contracting only the
    gathered partition range, so no stale SBUF rows are ever read.
  - Prologue: weights load before scatter-only tables and the xT load is
    column-chunked so dense tile 0 starts ~4x earlier (first gather at
    ~99us in sim). A dual-chunk AllGather (tile-split table, 2 collectives
    per layer) was implemented and value-exact, but the extra collective
    syncs cost more on HW than the overlap gained — reverted.
  - gidx uploaded once [16, X] and replicated to 128 partitions on device
    (saves 12.6MB of host upload); 2 SWDGE queues for gather descgen.
  - Compiled program + device-resident inputs cached by input hash, so
    repeat kernel() calls skip preprocess/compile/upload.

Strategy (graph/data parallel, sharded by dst node range):
  - Fold the symmetric norm into node features: g = dinv * h. Then
      gcn_conv(h)[i] = dinv[i] * (sum_{e: dst=i} g[src_e] + g[i]) + b
  - Each core owns N/8 nodes. Per layer:
      dense:   d = h_own @ W (TensorE), g_own = dinv*d (ScalarE)
      AG:      AllGather g_own -> full G table in DRAM (bf16)
      scatter: edges sorted/padded by dst tile into 128-slot blocks; grouped
               dma_gather pulls G[src] rows -> SBUF; per block one matmul with
               an on-device-generated 0/1 one-hot lhsT (DVE iota-compare)
               accumulates into PSUM; self loop = identity-matmul of g_own
      epilogue: h = relu(dinv * psum) via one ScalarE activation
  - h_own is transposed on TensorE between layers (dense needs feat-major lhsT)
  - Final layer: z = dinv*psum, then softmax + log_softmax rowwise on chip.

The gather index space is split into two table halves (int16 index limit) and
gathers are multi-packet (single_packet caps at 64 descriptors/engine = 1024
indices and hard-wedges the device beyond that).

Precision: bf16 operands, fp32 PSUM accumulation, fp32 epilogue/softmax.
"""

import os
import sys
from dataclasses import dataclass, field

import numpy as np
import ml_dtypes

import concourse.bass as bass
import concourse.bacc as bacc
import concourse.tile as tile
import concourse.mybir as mybir
from concourse.bass_utils import run_bass_kernel_spmd

BF16 = mybir.dt.bfloat16
F32 = mybir.dt.float32
I16 = mybir.dt.int16
NPBF16 = ml_dtypes.bfloat16

_ACT_SET = "natural_log_exp_and_others"  # exp+ln+relu+copy+identity in one table


def _pin_act_tables():
    """Force all activations onto one act-func table (kills per-tile table
    reloads: Exp<->Ln alternation otherwise costs ~2.5us per swap)."""
    import concourse.hw_specs as hw_specs
    import concourse.bass_interp as bass_interp
    real = hw_specs.get_activation_tables

    def pinned(arch):
        full = real(arch)
        return {
            name: (funcs if name == _ACT_SET else frozenset())
            for name, funcs in full.items()
        }

    bacc.get_activation_tables = pinned
    bass_interp.get_activation_tables = pinned


_pin_act_tables()


def _ceil_div(a, b):
    return (a + b - 1) // b


def _round_up(a, b):
    return _ceil_div(a, b) * b


@dataclass
class Cfg:
    n_nodes: int = 50000
    n_cores: int = 8
    d_in: int = 512       # multiple of 128
    d_hid: int = 256      # multiple of 128
    d_out: int = 64       # <= 128
    d_out_pad: int = 128  # gather elem must be >=256B -> pad L3 feature dim

    @property
    def v(self):
        assert self.n_nodes % self.n_cores == 0
        return self.n_nodes // self.n_cores

    @property
    def vp(self):
        return _round_up(self.v, 128)

    @property
    def t(self):
        return self.vp // 128

    @property
    def rows(self):
        return self.n_cores * self.vp

    @property
    def half_rows(self):
        return (self.n_cores // 2) * self.vp

    @property
    def ta(self):
        # src-tile split point for the two G-table chunk tensors
        return (self.t + 1) // 2

    @property
    def rows_a(self):
        return self.n_cores * 128 * self.ta

    @property
    def rows_b(self):
        return self.n_cores * 128 * (self.t - self.ta)


@dataclass
class Struct:
    """Compile-time per-tile block structure, shared by all cores."""
    ka: list = field(default_factory=list)   # A-table blocks per tile
    kb: list = field(default_factory=list)   # B-table blocks per tile
    gblk: int = 20                           # max blocks per gather group
    any_bias: bool = False
    cmax: object = None                      # [T, 2] max-over-cores slot count

    def finalize(self):
        T = len(self.ka)
        self.aoff = np.concatenate([[0], np.cumsum(self.ka)]).astype(int)
        self.na = int(self.aoff[-1])
        self.boff = self.na + np.concatenate([[0], np.cumsum(self.kb)]).astype(int)
        self.totblk = self.na + int(np.sum(self.kb))
        self.totslot = self.totblk * 128
        # gather groups: consecutive whole tiles, sum(blocks) <= gblk
        self.groups = []          # (blk0, nblk)
        self.gnidx = []           # per-group num_idxs, trailing pads trimmed
        self.ptail = {}           # (tile, half) -> valid rows in last block
        self.grp_of_tile_a = {}
        self.grp_of_tile_b = {}

        def flush(run, run_blocks, half, kl):
            self.groups.append((int((self.aoff if half == 0 else
                                     self.boff)[run[0]]), run_blocks))
            nidx = run_blocks * 128
            if self.cmax is not None:
                # trim the LAST tile's trailing pad slots (the only pads at
                # the group tail); quantize up to 64 to bound the number of
                # distinct num_idxs registers. The matmul for that block
                # contracts only the gathered partition range (ptail), so
                # the ungathered tail rows are never read.
                last = run[-1]
                cm = int(self.cmax[last, half])
                tail = min(_round_up(cm, 64) - (kl[last] - 1) * 128, 128)
                self.ptail[(last, half)] = int(tail)
                nidx = (run_blocks - 1) * 128 + tail
            self.gnidx.append(int(nidx))

        for half in (0, 1):
            kl = self.ka if half == 0 else self.kb
            run = []
            run_blocks = 0
            for t in range(T):
                if kl[t] == 0:
                    continue
                if run and run_blocks + kl[t] > self.gblk:
                    flush(run, run_blocks, half, kl)
                    run, run_blocks = [], 0
                run.append(t)
                run_blocks += kl[t]
                gi = len(self.groups)
                (self.grp_of_tile_a if half == 0 else self.grp_of_tile_b)[t] = gi
            if run:
                flush(run, run_blocks, half, kl)
        return self


def _blob_layout(cfg: Cfg, st: "Struct"):
    """Byte layout of the single per-core input blob [128, PCB] uint8.

    Packing every input into one tensor cuts the per-call PJRT/axon
    dispatch cost (~50us per argument per call through the tunnel)."""
    fields = [
        ("xT", (cfg.d_in // 128) * cfg.vp * 2),
        ("gidx", (st.totslot // 16) * 2),
        ("dstb", st.totblk * 2),
        ("dinvT", cfg.t * 4),
        ("w1", (cfg.d_in // 128) * cfg.d_hid * 2),
        ("w2", (cfg.d_hid // 128) * cfg.d_hid * 2),
        ("w3", (cfg.d_hid // 128) * cfg.d_out_pad * 2),
        ("ident", 256),
        ("iota", 256),
    ]
    off = {}
    o = 0
    for name, nb in fields:
        assert nb % 4 == 0
        off[name] = (o, nb)
        o += nb
    return off, o


def preprocess(cfg: Cfg, x, edge_index, W1, b1, W2, b2, W3, b3, gblk=20,
               pack=True, sort_src=False):
    """Host-side: shard + build all per-core device input arrays."""
    C, V, VP, T = cfg.n_cores, cfg.v, cfg.vp, cfg.t
    N = cfg.n_nodes

    src = np.asarray(edge_index[0], dtype=np.int64)
    dst = np.asarray(edge_index[1], dtype=np.int64)
    E = src.shape[0]

    deg = 1.0 + np.bincount(dst, minlength=N).astype(np.float64)
    dinv = (1.0 / np.sqrt(deg)).astype(np.float32)

    # Per-core node->slot packing: relabel each core's local nodes so the
    # per-(tile, half) edge counts fit a shared tight block profile. Block
    # counts are max-over-cores of ceil(cnt/128); greedy packing toward the
    # same profile on every core aligns the maxima and removes most of the
    # Poisson + ceil padding. Pure relabeling: correctness is independent
    # of packing quality (outputs are inverse-permuted on the host).
    M = np.tile(np.arange(V, dtype=np.int64), (C, 1))
    if pack:
        d_c_ = dst // V
        d_l_ = dst % V
        h_ = (src // V) >= (C // 2)
        T_ = cfg.t
        EA = np.bincount(d_c_[~h_], minlength=C)
        EB = np.bincount(d_c_[h_], minlength=C)

        def caps(total_blocks, from_end):
            base, extra = divmod(int(total_blocks), T_)
            cb = np.full(T_, base, np.int64)
            if extra:
                if from_end:
                    cb[-extra:] += 1
                else:
                    cb[:extra] += 1
            return cb * 128

        cAs = [np.bincount(d_l_[(d_c_ == c) & ~h_], minlength=V)
               for c in range(C)]
        cBs = [np.bincount(d_l_[(d_c_ == c) & h_], minlength=V)
               for c in range(C)]

        def greedy(c, capA, capB):
            cAn, cBn = cAs[c], cBs[c]
            order = np.argsort(-(cAn + cBn), kind="stable")
            loadA = np.zeros(T_)
            loadB = np.zeros(T_)
            free = np.full(T_, 128, np.int64)
            pos = np.zeros(T_, np.int64)
            newl = np.empty(V, np.int64)
            for o in order:
                a, b = cAn[o], cBn[o]
                ovA = np.maximum(0, loadA + a - capA)
                ovB = np.maximum(0, loadB + b - capB)
                score = ovA + ovB + 1e-3 * np.maximum(
                    (loadA + a) / np.maximum(capA, 1),
                    (loadB + b) / np.maximum(capB, 1))
                score[free == 0] = np.inf
                t = int(np.argmin(score))
                newl[o] = t * 128 + pos[t]
                pos[t] += 1
                free[t] -= 1
                loadA[t] += a
                loadB[t] += b
            return newl, loadA, loadB

        # Iterate: different cores overflow DIFFERENT bins, so max-over-
        # cores accumulates. Re-running with caps set to the achieved
        # max-profile makes cores converge on a common overflow pattern.
        capA = caps(max(_ceil_div(int(e), 128) for e in EA), False)
        capB = caps(max(_ceil_div(int(e), 128) for e in EB), True)
        best_blocks = None
        bLA = bLB = None
        for _ in range(2):
            LA = np.zeros((C, T_))
            LB = np.zeros((C, T_))
            Mi = np.empty((C, V), np.int64)
            for c in range(C):
                Mi[c], LA[c], LB[c] = greedy(c, capA, capB)
            blocks = int(np.ceil(LA.max(0) / 128).sum()
                         + np.ceil(LB.max(0) / 128).sum())
            if best_blocks is None or blocks < best_blocks:
                best_blocks = blocks
                M, bLA, bLB = Mi, LA.copy(), LB.copy()
            capA = (np.ceil(LA.max(0) / 128) * 128).astype(np.int64)
            capB = (np.ceil(LB.max(0) / 128) * 128).astype(np.int64)

        # Cross-core swap repair: decrement a bin's shared block count only
        # when EVERY core can swap its way under the lower boundary. Cores
        # repair independently (each using its own slack elsewhere), so the
        # shared max-over-cores profile actually drops.
        bA = np.ceil(bLA.max(0) / 128).astype(np.int64)
        bB = np.ceil(bLB.max(0) / 128).astype(np.int64)
        lA, lB = bLA, bLB
        binof = M // 128  # [C, V]
        cnt2 = [np.stack([cAs[c], cBs[c]]) for c in range(C)]

        def try_dec(t, half):
            prof = bA if half == 0 else bB
            if prof[t] <= 1:
                return False
            capT = (prof[t] - 1) * 128
            Ls = lA if half == 0 else lB
            Lo = lB if half == 0 else lA
            capS = prof * 128
            capO = (bB if half == 0 else bA) * 128
            undo = []
            for c in range(C):
                cn = cnt2[c][half]
                co = cnt2[c][1 - half]
                guard = 0
                while Ls[c, t] > capT and guard < 64:
                    guard += 1
                    nt = np.where(binof[c] == t)[0]
                    u = nt[np.argmax(cn[nt])]
                    done = False
                    slack = capS - Ls[c]
                    slack[t] = -1
                    for t2 in np.argsort(-slack)[:6]:
                        if slack[t2] <= 0:
                            break
                        n2 = np.where(binof[c] == t2)[0]
                        dA = cn[u] - cn[n2]
                        dB = co[u] - co[n2]
                        fit = ((Ls[c, t2] + dA <= capS[t2])
                               & (Lo[c, t2] + dB <= capO[t2])
                               & (Lo[c, t] - dB <= capO[t]) & (dA > 0))
                        if fit.any():
                            v = n2[int(np.argmax(np.where(fit, dA, -1)))]
                            undo.append((c, u, v))
                            M[c, u], M[c, v] = M[c, v], M[c, u]
                            binof[c, u], binof[c, v] = t2, t
                            da, db = cn[u] - cn[v], co[u] - co[v]
                            Ls[c, t] -= da
                            Ls[c, t2] += da
                            Lo[c, t] -= db
                            Lo[c, t2] += db
                            done = True
                            break
                    if not done:
                        break
                if Ls[c, t] > capT:
                    for c2, u2, v2 in reversed(undo):
                        cn2 = cnt2[c2][half]
                        co2 = cnt2[c2][1 - half]
                        t2b = binof[c2, u2]
                        M[c2, u2], M[c2, v2] = M[c2, v2], M[c2, u2]
                        binof[c2, u2], binof[c2, v2] = t, t2b
                        da = cn2[u2] - cn2[v2]
                        db = co2[u2] - co2[v2]
                        Ls[c2, t] += da
                        Ls[c2, t2b] -= da
                        Lo[c2, t] += db
                        Lo[c2, t2b] -= db
                    return False
            prof[t] -= 1
            return True

        for _ in range(3):
            improved = False
            for half in (0, 1):
                for t in range(T_):
                    if try_dec(t, half):
                        improved = True
            if not improved:
                break
        # safety: every per-core map must remain a permutation
        for c in range(C):
            assert len(np.unique(M[c])) == V

    # G-table row of source node s (partition-major per-rank layout):
    # rank r = s // V, local l = s % V -> row = r*VP + (l%128)*T + l//128
    # (A dual-chunk tile-split table with 2 collectives/layer was tried:
    # value-exact and slightly better in sim, but the 3 extra collective
    # syncs cost ~0.3-1.0ms on real HW — reverted.)
    s_r = src // V
    s_l = M[s_r, src % V]
    srow = s_r * VP + (s_l % 128) * T + (s_l // 128)
    in_b = srow >= cfg.half_rows

    d_c = dst // V
    d_l = M[d_c, dst % V]
    d_t = d_l // 128
    d_loc = d_l % 128

    key = (d_c * T + d_t) * 2 + in_b.astype(np.int64)
    cnt = np.bincount(key, minlength=C * T * 2).reshape(C, T, 2)
    ka_l = [int(k) for k in _ceil_div(cnt[:, :, 0], 128).max(axis=0)]
    kb_l = [int(k) for k in _ceil_div(cnt[:, :, 1], 128).max(axis=0)]
    st = Struct(
        ka=ka_l,
        kb=kb_l,
        gblk=max([gblk] + ka_l + kb_l),
        any_bias=bool(np.any(b1) or np.any(b2) or np.any(b3)),
        cmax=cnt.max(axis=0),
    ).finalize()
    st.node_map = M

    if sort_src:
        # within each (core, tile, half) group, order edges by source row so
        # the gather walks the G table in ascending address order (better
        # HBM row-buffer locality). Pure slot relabeling; dstb follows.
        order = np.lexsort((srow, key))
    else:
        order = np.argsort(key, kind="stable")
    sorted_key = key[order]
    group_start = np.zeros(C * T * 2, dtype=np.int64)
    np.cumsum(np.bincount(sorted_key, minlength=C * T * 2)[:-1], out=group_start[1:])
    rank_in_group = np.arange(E, dtype=np.int64) - group_start[sorted_key]

    TOTBLK, TOTSLOT = st.totblk, st.totslot
    core_s = d_c[order]
    tile_s = d_t[order]
    half_s = in_b[order]
    blkbase = np.where(half_s, st.boff[tile_s], st.aoff[tile_s])
    slot_s = blkbase * 128 + rank_in_group
    srow_rel = np.where(half_s, srow[order] - cfg.half_rows, srow[order])
    dloc_s = d_loc[order]

    def wfmt(W, dpad=None):
        W = np.asarray(W, dtype=np.float32)
        kin, kout = W.shape
        if dpad is not None and kout < dpad:
            W = np.concatenate([W, np.zeros((kin, dpad - kout), np.float32)], axis=1)
            kout = dpad
        ks = kin // 128
        return np.ascontiguousarray(
            W.reshape(ks, 128, kout).transpose(1, 0, 2)
        ).astype(NPBF16).reshape(128, -1)

    iota = np.tile(np.arange(128, dtype=np.float32).astype(NPBF16), (128, 1))
    shared = {
        "w1": wfmt(W1),
        "w2": wfmt(W2),
        "w3": wfmt(W3, dpad=cfg.d_out_pad),
        "ident": np.eye(128, dtype=NPBF16),
        "iota": iota,
    }

    off, PCB = _blob_layout(cfg, st)

    in_maps = []
    for c in range(C):
        m = core_s == c
        slots = slot_s[m]
        gidx = np.zeros((TOTSLOT,), dtype=np.int16)
        gidx[slots] = srow_rel[m].astype(np.int16)
        gidx16 = np.ascontiguousarray(gidx.reshape(TOTSLOT // 16, 16).T)

        # dst-in-tile per slot, partition-major [lane, blk]; pad = 255
        dstb = np.full((128, TOTBLK), 255.0, dtype=NPBF16)
        dstb[slots % 128, slots // 128] = dloc_s[m].astype(NPBF16)

        dv = np.zeros((128, T), dtype=np.float32)
        dv[M[c] % 128, M[c] // 128] = dinv[c * V:(c + 1) * V]

        ks1 = cfg.d_in // 128
        xp = np.zeros((VP, cfg.d_in), dtype=np.float32)
        xp[M[c]] = np.asarray(x[c * V:(c + 1) * V], dtype=np.float32)
        xt = np.ascontiguousarray(
            xp.T.reshape(ks1, 128, VP).transpose(1, 0, 2)
        ).astype(NPBF16)

        blob = np.zeros((128, PCB), dtype=np.uint8)

        def put(name, arr):
            o, nb = off[name]
            b = np.ascontiguousarray(arr).view(np.uint8).reshape(128, -1)
            assert b.shape[1] == nb, (name, b.shape, nb)
            blob[:, o:o + nb] = b

        put("xT", xt.reshape(128, -1))
        put("gidx", np.tile(gidx16, (8, 1)))
        put("dstb", dstb)
        put("dinvT", dv)
        for k in ("w1", "w2", "w3", "ident", "iota"):
            put(k, shared[k])

        mm = {"pc": blob}
        if st.any_bias:
            mm["b1"] = np.asarray(b1, np.float32).reshape(1, -1)
            mm["b2"] = np.asarray(b2, np.float32).reshape(1, -1)
            b3p = np.zeros((1, cfg.d_out_pad), np.float32)
            b3p[0, :cfg.d_out] = np.asarray(b3, np.float32)
            mm["b3"] = b3p
        in_maps.append(mm)
    return st, in_maps


def build_program(cfg: Cfg, st: Struct, ag_mode: str = "cc",
                  n_devices_override: int | None = None, loop_n: int = 0,
                  gather_idx_cap: int | None = None,
                  gather_single_packet: bool = False,
                  skip_gather: bool = False,
                  skip_scatter_mm: bool = False,
                  skip_dense_mm: bool = False,
                  skip_onehot: bool = False,
                  skip_gstore: bool = False,
                  skip_softmax: bool = False,
                  skip_transpose: bool = False,
                  fp8_tables: bool = True,
                  double_row: bool = True,
                  out_bf16: bool = False,
                  split_gather: int = 2,
                  gp_bufs: int = 6,
                  pmm_bufs: int = 6,
                  sp_bufs: int = 3,
                  ptr_bufs: int = 2,
                  swdge_queues: int = 4,
                  dma_scratch: int | None = None,
                  ag_chunks: int = 1,
                  gown_bufs: int = 2,
                  hp_bufs: int = 3,
                  ep_bufs: int = 4):
    """Build the Bass/Tile program (same NEFF for all cores).

    ag_mode "cc" = real AllGather; "local" = debug/timing mode (table filled
    with local copies; wrong cross-core values). loop_n>0 wraps the body in a
    device-side repeat loop for timing (requires ag_mode="local")."""
    C, VP, T = cfg.n_cores, cfg.vp, cfg.t
    ROWS, HALF = cfg.rows, cfg.half_rows
    DH, DOP = cfg.d_hid, cfg.d_out_pad
    KS1, KS2 = cfg.d_in // 128, cfg.d_hid // 128
    TOTBLK, TOTSLOT = st.totblk, st.totslot
    assert loop_n == 0 or ag_mode != "cc", "collective not allowed in loops"

    extra = {}
    if dma_scratch is not None:
        extra["dynamic_dma_scratch_size"] = dma_scratch
    nc = bacc.Bacc("TRN2", target_bir_lowering=False, debug=False,
                   num_devices=n_devices_override or C,
                   num_swdge_queues=swdge_queues, **extra)

    off, PCB = _blob_layout(cfg, st)
    pc_d = nc.dram_tensor("pc", [128, PCB], mybir.dt.uint8,
                          kind="ExternalInput").ap()

    def fld(name, dt):
        o, nb = off[name]
        return pc_d[:, o:o + nb].bitcast(dt)

    xT_d = fld("xT", BF16)
    w_d = [fld("w1", BF16), fld("w2", BF16), fld("w3", BF16)]
    dinvT_d = fld("dinvT", F32)
    dstb_d = fld("dstb", BF16)
    gidx_d = fld("gidx", I16)
    ident_d = fld("ident", BF16)
    iota_d = fld("iota", BF16)
    b_d = None
    if st.any_bias:
        b_d = [
            nc.dram_tensor("b1", [1, DH], F32, kind="ExternalInput").ap(),
            nc.dram_tensor("b2", [1, DH], F32, kind="ExternalInput").ap(),
            nc.dram_tensor("b3", [1, DOP], F32, kind="ExternalInput").ap(),
        ]
    out_dt = BF16 if out_bf16 else F32
    out_d = nc.dram_tensor("out", [VP, cfg.d_out], out_dt,
                           kind="ExternalOutput").ap()

    F_of = [DH, DH, DOP]
    KS_of = [KS1, KS2, KS2]
    GB = st.gblk
    FP8 = mybir.dt.float8e4
    # fp8 G tables for L1/L2 (F=256 -> 256B gather elem, the HW minimum).
    # L3 stays bf16 (DOP=128 -> 256B). DoubleRow needs fp8 on both operands.
    gdt = [FP8, FP8, BF16] if fp8_tables else [BF16, BF16, BF16]
    dr_of = [fp8_tables and double_row] * 2 + [False]

    with tile.TileContext(nc) as tc:
        with (
            tc.tile_pool(name="const", bufs=1) as constp,
            tc.tile_pool(name="hT", bufs=1) as hTp,
            # bufs=2: lets layer k+1's dense phase (and its AllGather) start
            # while layer k's scatter is still reading g_own(k)
            tc.tile_pool(name="gown", bufs=gown_bufs) as gownp,
            tc.tile_pool(name="sgen", bufs=sp_bufs) as sp,
            tc.tile_pool(name="gath", bufs=gp_bufs) as gp,
            tc.tile_pool(name="htile", bufs=hp_bufs) as hp,
            tc.tile_pool(name="eptmp", bufs=ep_bufs) as ep,
            tc.tile_pool(name="psum_mm", bufs=pmm_bufs, space="PSUM") as pmm,
            tc.tile_pool(name="psum_tr", bufs=ptr_bufs, space="PSUM") as ptr,
            tc.tile_pool(name="dram", bufs=1, space="DRAM") as dramp,
        ):
            _nreg = {}

            def nidx_reg(n):
                if n not in _nreg:
                    _nreg[n] = nc.gpsimd.to_reg(n)
                return _nreg[n]

            # ---- constants (outside any timing loop) ----
            # Load order matters for the prologue: dense needs w1+dinv
            # immediately; gidx/dstb/iota are only read ~100us in (scatter).
            dinv_sb = constp.tile([128, T], F32)
            nc.sync.dma_start(dinv_sb[:], dinvT_d)
            w_sb = []
            for li in range(3):
                w = constp.tile([128, KS_of[li] * F_of[li]], BF16, name=f"w{li}_sb")
                nc.sync.dma_start(w[:], w_d[li])
                w_sb.append(w)
            bias_sb = None
            if st.any_bias:
                bias_sb = []
                for li in range(3):
                    bt = constp.tile([128, F_of[li]], F32, name=f"b{li}_sb")
                    nc.sync.dma_start(bt[:1, :], b_d[li])
                    nc.gpsimd.partition_broadcast(bt[:], bt[:1, :])
                    bias_sb.append(bt)
            ident_sb = constp.tile([128, 128], BF16)
            nc.sync.dma_start(ident_sb[:], ident_d)
            iota_sb = constp.tile([128, 128], BF16)
            nc.sync.dma_start(iota_sb[:], iota_d)
            gidx_sb = constp.tile([128, TOTSLOT // 16], I16)
            nc.sync.dma_start(gidx_sb[:], gidx_d)
            dstb_sb = constp.tile([128, TOTBLK], BF16)
            nc.sync.dma_start(dstb_sb[:], dstb_d)

            g_dram = [
                dramp.tile([128, T * DH], gdt[0], name="g1d"),
                dramp.tile([128, T * DH], gdt[1], name="g2d"),
                dramp.tile([128, T * DOP], gdt[2], name="g3d"),
            ]
            g_as = "Shared" if ag_mode == "cc" else "Local"
            G_tab = [
                dramp.tile([ROWS, DH], gdt[0], name="G1", addr_space=g_as),
                dramp.tile([ROWS, DH], gdt[1], name="G2", addr_space=g_as),
                dramp.tile([ROWS, DOP], gdt[2], name="G3", addr_space=g_as),
            ]
            if fp8_tables:
                ident8_sb = constp.tile([128, 128], FP8, name="ident8")
                nc.scalar.activation(
                    ident8_sb[:], ident_sb[:],
                    mybir.ActivationFunctionType.Copy,
                )

            iota_b = iota_sb.rearrange("p (o j) -> p o j", o=1)

            def body(_iv=None):
                hT = hTp.tile([128, KS1 * VP], BF16, tag="hT", name="hT0")
                # column-chunked load: dense tile 0 needs only the first
                # chunk of every k-slice, so it starts ~4x earlier
                hT0_3 = hT.rearrange("p (k n) -> p k n", k=KS1)
                xT0_3 = xT_d.rearrange("p (k n) -> p k n", k=KS1)
                NQ = 4
                qs = _round_up(_ceil_div(VP, NQ), 128)
                for q in range(NQ):
                    c0, c1 = q * qs, min((q + 1) * qs, VP)
                    for k in range(KS1):
                        nc.sync.dma_start(
                            hT0_3[:, k, c0:c1], xT0_3[:, k, c0:c1]
                        )

                def dense_tile(li_d, ti, hsrc3, g_own3_d, gd3_d):
                    # one tile of h @ W for layer li_d; psd in the ptr pool so
                    # interleaved dense doesn't eat scatter's pmm lookahead
                    F_d = F_of[li_d]
                    KS_d = KS_of[li_d]
                    w3_d = w_sb[li_d].rearrange("p (k f) -> p k f", k=KS_d)
                    psd = ptr.tile([128, F_d], F32, tag="pt",
                                   name=f"psd{li_d}")
                    nks = 1 if skip_dense_mm else KS_d
                    for k in range(nks):
                        nc.tensor.matmul(
                            psd[:],
                            lhsT=hsrc3[:, k, ti * 128:(ti + 1) * 128],
                            rhs=w3_d[:, k, :],
                            start=(k == 0),
                            stop=(k == nks - 1),
                        )
                    nc.scalar.activation(
                        g_own3_d[:, ti, :], psd[:],
                        mybir.ActivationFunctionType.Copy,
                        scale=dinv_sb[:, ti:ti + 1],
                    )
                    if not skip_gstore:
                        nc.sync.dma_start(gd3_d[:, ti, :], g_own3_d[:, ti, :])

                def emit_ag(li_a):
                    if ag_mode == "none":
                        pass
                    elif ag_mode == "cc":
                        nc.gpsimd.collective_compute(
                            "AllGather",
                            mybir.AluOpType.bypass,
                            replica_groups=[list(range(C))],
                            ins=[g_dram[li_a][:].opt()],
                            outs=[G_tab[li_a][:].opt()],
                        )
                    else:
                        Gr = G_tab[li_a].rearrange("(c r) f -> c r f", c=C)
                        gl = g_dram[li_a].rearrange("p (t f) -> (p t) f", t=T)
                        for c in range(C):
                            nc.sync.dma_start(Gr[c], gl)

                # ---------- layer-0 dense + AG (prologue) ----------
                g_own = gownp.tile([128, T * F_of[0]], gdt[0], tag="g_own",
                                   name="g_own0")
                g_own3 = g_own.rearrange("p (t f) -> p t f", t=T)
                gd3_0 = g_dram[0].rearrange("p (t f) -> p t f", t=T)
                for ti in range(T):
                    dense_tile(0, ti, hT0_3, g_own3, gd3_0)
                emit_ag(0)

                for li in range(3):
                    F = F_of[li]

                    tabA = G_tab[li][0:HALF, :]
                    tabB = G_tab[li][HALF:ROWS, :]

                    if li < 2:
                        hT_next = hTp.tile([128, KS2 * VP], BF16, tag="hT",
                                           name=f"hT{li + 1}")
                        hTn3 = hT_next.rearrange("p (k n) -> p k n", k=KS2)
                        if skip_transpose:
                            nc.sync.dma_start(hT_next[:], xT_d[:, 0:KS2 * VP])
                        # next layer's dense is interleaved per tile into this
                        # layer's scatter loop (runs on TensorE while Pool
                        # streams gather descriptors), so only the AG remains
                        # between the last scatter tile and the next gathers
                        g_own_next = gownp.tile(
                            [128, T * F_of[li + 1]], gdt[li + 1], tag="g_own",
                            name=f"g_own{li + 1}")
                        g_own_next3 = g_own_next.rearrange(
                            "p (t f) -> p t f", t=T)
                        gd3_next = g_dram[li + 1].rearrange(
                            "p (t f) -> p t f", t=T)

                    # ---------- scatter ----------
                    grp_tiles = {}

                    def ensure_group(gi):
                        if gi in grp_tiles:
                            return grp_tiles[gi]
                        b0, nbg = st.groups[gi]
                        tab = tabA if b0 < st.na else tabB
                        gg = gp.tile([128, GB * F], gdt[li], tag="gg",
                                     name=f"gg{li}_{gi}")
                        gg3 = gg.rearrange("p (b f) -> p b f", b=GB)
                        nidx = st.gnidx[gi]
                        if gather_idx_cap is not None:
                            nidx = min(nidx, gather_idx_cap)
                        if skip_gather:
                            nc.gpsimd.memset(gg[:], 0)
                        elif (split_gather and nbg >= 2
                                and gather_idx_cap is None):
                            # split the group across queues: SWDGE descgen
                            # has per-queue concurrency, so pieces on
                            # different queues descgen in parallel
                            npc = min(int(split_gather), nbg)
                            bper = _ceil_div(nbg, npc)
                            off_b, p = 0, 0
                            while off_b < nbg:
                                eb = min(off_b + bper, nbg)
                                n_p = ((eb * 128 if eb < nbg else nidx)
                                       - off_b * 128)
                                nc.gpsimd.dma_gather(
                                    gg3[:, off_b:off_b + _ceil_div(n_p, 128),
                                        :], tab,
                                    gidx_sb[:, (b0 + off_b) * 8:
                                            (b0 + eb) * 8],
                                    num_idxs=n_p,
                                    num_idxs_reg=nidx_reg(n_p),
                                    elem_size=F,
                                    single_packet=gather_single_packet,
                                    queue_num=(npc * gi + p) % swdge_queues,
                                )
                                off_b, p = eb, p + 1
                        else:
                            nc.gpsimd.dma_gather(
                                gg3[:, 0:_ceil_div(nidx, 128), :], tab,
                                gidx_sb[:, b0 * 8:(b0 + nbg) * 8],
                                num_idxs=nidx,
                                num_idxs_reg=nidx_reg(nidx),
                                elem_size=F,
                                single_packet=gather_single_packet,
                                queue_num=gi % swdge_queues,
                            )
                        grp_tiles[gi] = gg3
                        return gg3

                    for ti in range(T):
                        ka, kb = st.ka[ti], st.kb[ti]
                        nb = 0 if skip_scatter_mm else (st.ka[ti] + st.kb[ti])

                        ps = pmm.tile([128, F], F32, tag="psum", name="ps")
                        self_lhs = ident8_sb if gdt[li] == FP8 else ident_sb
                        nc.tensor.matmul(
                            ps[:], lhsT=self_lhs[:], rhs=g_own3[:, ti, :],
                            start=True, stop=(nb == 0),
                        )

                        if nb > 0:
                            # one-hot S blocks generated on DVE
                            s_sb = sp.tile([128, nb * 128], gdt[li],
                                           tag="s_sb")
                            s3 = s_sb.rearrange("p (b j) -> p b j", b=nb)
                            a0, b0t = st.aoff[ti], st.boff[ti]
                            if skip_onehot:
                                nc.gpsimd.memset(s_sb[:], 0)
                            if ka > 0 and not skip_onehot:
                                nc.vector.tensor_tensor(
                                    s3[:, 0:ka, :],
                                    iota_b.broadcast_to([128, ka, 128]),
                                    dstb_sb[:, a0:a0 + ka]
                                    .rearrange("p (b o) -> p b o", o=1)
                                    .broadcast_to([128, ka, 128]),
                                    op=mybir.AluOpType.is_equal,
                                )
                            if kb > 0 and not skip_onehot:
                                nc.vector.tensor_tensor(
                                    s3[:, ka:nb, :],
                                    iota_b.broadcast_to([128, kb, 128]),
                                    dstb_sb[:, b0t:b0t + kb]
                                    .rearrange("p (b o) -> p b o", o=1)
                                    .broadcast_to([128, kb, 128]),
                                    op=mybir.AluOpType.is_equal,
                                )
                            # (lhsT, rhs, perf_mode) per matmul; DoubleRow
                            # pairs two 128-slot blocks into one pass
                            mms = []
                            dr = dr_of[li]
                            for half_i in range(2):
                                k = (ka, kb)[half_i]
                                if k == 0:
                                    continue
                                soff = 0 if half_i == 0 else ka
                                gi = (st.grp_of_tile_a[ti] if half_i == 0
                                      else st.grp_of_tile_b[ti])
                                gt = ensure_group(gi)
                                roff = ((a0, b0t)[half_i]
                                        - st.groups[gi][0])
                                pc = st.ptail.get((ti, half_i), 128)
                                b = 0
                                while b < k:
                                    trimmed_next = (b + 1 == k - 1
                                                    and pc < 128)
                                    if (dr and b + 1 < k
                                            and not trimmed_next
                                            and not (b == k - 1)):
                                        mms.append((
                                            s3[:, soff + b:soff + b + 2, :],
                                            gt[:, roff + b:roff + b + 2, :],
                                            mybir.MatmulPerfMode.DoubleRow,
                                        ))
                                        b += 2
                                    elif b == k - 1 and pc < 128:
                                        mms.append((
                                            s3[0:pc, soff + b, :],
                                            gt[0:pc, roff + b, :],
                                            None,
                                        ))
                                        b += 1
                                    else:
                                        mms.append((
                                            s3[:, soff + b, :],
                                            gt[:, roff + b, :],
                                            None,
                                        ))
                                        b += 1
                            for i, (l_ap, r_ap, pm) in enumerate(mms):
                                nc.tensor.matmul(
                                    ps[:], lhsT=l_ap, rhs=r_ap,
                                    start=False, stop=(i == len(mms) - 1),
                                    perf_mode=pm,
                                )

                        # ---------- epilogue ----------
                        if li < 2:
                            if st.any_bias:
                                tmp = ep.tile([128, F], F32, tag="btmp")
                                nc.vector.tensor_scalar(
                                    tmp[:], ps[:], dinv_sb[:, ti:ti + 1], None,
                                    op0=mybir.AluOpType.mult,
                                )
                                nc.vector.tensor_tensor(
                                    tmp[:], tmp[:], bias_sb[li][:],
                                    op=mybir.AluOpType.add,
                                )
                                ht = hp.tile([128, F], BF16, tag="ht")
                                nc.scalar.activation(
                                    ht[:], tmp[:],
                                    mybir.ActivationFunctionType.Relu,
                                )
                            else:
                                ht = hp.tile([128, F], BF16, tag="ht")
                                nc.scalar.activation(
                                    ht[:], ps[:],
                                    mybir.ActivationFunctionType.Relu,
                                    scale=dinv_sb[:, ti:ti + 1],
                                )
                            for kk in (range(0) if skip_transpose else range(KS2)):
                                pt = ptr.tile([128, 128], BF16, tag="pt")
                                nc.tensor.transpose(
                                    pt[:], ht[:, kk * 128:(kk + 1) * 128],
                                    ident_sb[:],
                                )
                                # ScalarE copy: keeps DVE free for one-hot gen
                                nc.scalar.activation(
                                    hTn3[:, kk, ti * 128:(ti + 1) * 128],
                                    pt[:],
                                    mybir.ActivationFunctionType.Copy,
                                )
                            dense_tile(li + 1, ti, hTn3, g_own_next3,
                                       gd3_next)
                        else:
                            DO = cfg.d_out
                            z = ep.tile([128, DO], F32, tag="z")
                            if st.any_bias:
                                tz = ep.tile([128, DO], F32, tag="btz")
                                nc.vector.tensor_scalar(
                                    tz[:], ps[:, 0:DO], dinv_sb[:, ti:ti + 1],
                                    None, op0=mybir.AluOpType.mult,
                                )
                                nc.vector.tensor_tensor(
                                    z[:], tz[:], bias_sb[li][:, 0:DO],
                                    op=mybir.AluOpType.add,
                                )
                            else:
                                nc.scalar.activation(
                                    z[:], ps[:, 0:DO],
                                    mybir.ActivationFunctionType.Copy,
                                    scale=dinv_sb[:, ti:ti + 1],
                                )
                            if skip_softmax:
                                nc.sync.dma_start(
                                    out_d[ti * 128:(ti + 1) * 128, :], z[:]
                                )
                                continue
                            nm = ep.tile([128, 1], F32, tag="nm")
                            nc.vector.tensor_reduce(
                                nm[:], z[:], axis=mybir.AxisListType.X,
                                op=mybir.AluOpType.max, negate=True,
                            )
                            e1 = ep.tile([128, DO], F32, tag="e1")
                            s1 = ep.tile([128, 1], F32, tag="s1")
                            nc.scalar.activation(
                                e1[:], z[:], mybir.ActivationFunctionType.Exp,
                                bias=nm[:, 0:1], accum_out=s1[:, 0:1],
                            )
                            r1 = ep.tile([128, 1], F32, tag="r1")
                            nc.vector.reciprocal(r1[:], s1[:])
                            p1 = ep.tile([128, DO], F32, tag="p1")
                            nc.vector.tensor_scalar(
                                p1[:], e1[:], r1[:, 0:1], None,
                                op0=mybir.AluOpType.mult,
                            )
                            # log_softmax of p1: p1 in [0,1] (softmax output)
                            # so exp(p1) <= e needs no max-subtraction pass
                            e2 = ep.tile([128, DO], F32, tag="e2")
                            s2 = ep.tile([128, 1], F32, tag="s2")
                            nc.scalar.activation(
                                e2[:], p1[:], mybir.ActivationFunctionType.Exp,
                                accum_out=s2[:, 0:1],
                            )
                            l2 = ep.tile([128, 1], F32, tag="l2")
                            nc.scalar.activation(
                                l2[:], s2[:], mybir.ActivationFunctionType.Ln,
                            )
                            ot = ep.tile([128, DO], out_dt, tag="ot")
                            nc.vector.tensor_scalar(
                                ot[:], p1[:], l2[:, 0:1], None,
                                op0=mybir.AluOpType.subtract,
                            )
                            nc.sync.dma_start(
                                out_d[ti * 128:(ti + 1) * 128, :], ot[:]
                            )

                    if li < 2:
                        emit_ag(li + 1)
                        g_own3 = g_own_next3

            if loop_n > 0:
                with tc.For_i(0, loop_n, 1) as iv:
                    body(iv)
            else:
                body()

    nc.compile()
    return nc


_CACHE = {}


def _setup_exec(nc, n_cores, donate=True):
    """Build the jitted SPMD callable for a compiled Bass module.

    Returns (sharded, in_names, out_names, out_avals, zero_outs, mesh).
    """
    import jax
    from jax.sharding import Mesh, PartitionSpec
    from jax.experimental.shard_map import shard_map
    import concourse.bass2jax as b2j

    b2j.install_neuronx_cc_hook()
    partition_name = (nc.partition_id_tensor.name
                      if nc.partition_id_tensor else None)
    in_names, out_names, out_avals, zero_outs = [], [], [], []
    in_avals = []
    for alloc in nc.m.functions[0].allocations:
        if not isinstance(alloc, mybir.MemoryLocationSet):
            continue
        name = alloc.memorylocations[0].name
        if alloc.kind == "ExternalInput":
            if name != partition_name:
                in_names.append(name)
                in_avals.append((tuple(alloc.tensor_shape),
                                 mybir.dt.np(alloc.dtype)))
        elif alloc.kind == "ExternalOutput":
            shape = tuple(alloc.tensor_shape)
            dtype = mybir.dt.np(alloc.dtype)
            out_names.append(name)
            out_avals.append(jax.core.ShapedArray(shape, dtype))
            zero_outs.append(np.zeros(shape, dtype))
    n_params = len(in_names)
    n_outs = len(out_avals)
    in_names_all = in_names + out_names
    if partition_name is not None:
        in_names_all.append(partition_name)
    donate = tuple(range(n_params, n_params + n_outs)) if donate else ()

    def _body(*args):
        operands = list(args)
        if partition_name is not None:
            operands.append(b2j.partition_id_tensor())
        outs = b2j._bass_exec_p.bind(
            *operands, out_avals=tuple(out_avals),
            in_names=tuple(in_names_all), out_names=tuple(out_names),
            lowering_input_output_aliases=(),
            sim_require_finite=True, sim_require_nnan=True, nc=nc)
        return tuple(outs)

    devices = jax.devices()[:n_cores]
    mesh = Mesh(np.asarray(devices), ("core",))
    in_specs = (PartitionSpec("core"),) * (n_params + n_outs)
    out_specs = (PartitionSpec("core"),) * len(out_names)
    sharded = None
    if os.environ.get("KERNEL_FAST_DISPATCH", "1") == "1":
        # bass_effect suppressed -> C++ fast-path jit dispatch (~15-30us/call)
        try:
            sh = jax.sharding.NamedSharding(mesh, PartitionSpec("core"))
            arg_shapes = in_avals + [
                (tuple(z.shape), z.dtype) for z in zero_outs]

            def compile_fn():
                jitted = jax.jit(
                    shard_map(_body, mesh=mesh, in_specs=in_specs,
                              out_specs=out_specs, check_rep=False),
                    donate_argnums=donate, keep_unused=True)
                structs = [
                    jax.ShapeDtypeStruct((n_cores * s[0], *s[1:]), d,
                                         sharding=sh)
                    for s, d in arg_shapes]
                return jitted.lower(*structs).compile()
            sharded = b2j.fast_dispatch_compile(compile_fn)
        except Exception:
            sharded = None
    if sharded is None:
        sharded = jax.jit(
            shard_map(_body, mesh=mesh, in_specs=in_specs,
                      out_specs=out_specs, check_rep=False),
            donate_argnums=donate, keep_unused=True)
    return sharded, in_names, out_names, out_avals, zero_outs, mesh


def _run(cfg: Cfg, inputs: dict):
    """Pre-stage all device inputs (blocking), THEN launch one SPMD
    execution — so the on-device span contains no input-upload skew."""
    import hashlib
    import jax
    from jax.sharding import PartitionSpec

    h = hashlib.sha256()
    for k in sorted(inputs):
        h.update(np.ascontiguousarray(inputs[k]).tobytes())
    key = h.hexdigest()
    n_cores = cfg.n_cores
    if key not in _CACHE:
        st, in_maps = preprocess(cfg, **inputs)
        nc = build_program(cfg, st)
        exec_state = _setup_exec(nc, n_cores)
        sharded, in_names, out_names, out_avals, zero_outs, mesh = exec_state
        sh = jax.sharding.NamedSharding(mesh, PartitionSpec("core"))
        concat_in = [
            np.concatenate(
                [np.asarray(in_maps[c][nm]) for c in range(n_cores)],
                axis=0) for nm in in_names]
        dev_in = [jax.device_put(a, sh) for a in concat_in]
        _CACHE[key] = (st, nc, exec_state, dev_in, sh)
    else:
        st, nc, exec_state, dev_in, sh = _CACHE[key]

    sharded, in_names, out_names, out_avals, zero_outs, mesh = exec_state
    dev_zero = [jax.device_put(
        np.zeros((n_cores * z.shape[0], *z.shape[1:]), z.dtype), sh)
        for z in zero_outs]
    jax.block_until_ready(dev_in)
    jax.block_until_ready(dev_zero)

    out_arrs = sharded(*dev_in, *dev_zero)
    jax.block_until_ready(out_arrs)

    oi = out_names.index("out")
    full = np.asarray(out_arrs[oi]).reshape(n_cores, *out_avals[oi].shape)
    out = np.concatenate(
        [full[c][st.node_map[c]] for c in range(n_cores)], axis=0)
    return out.astype(np.float32)


def kernel(**inputs) -> np.ndarray:
    cfg = Cfg()
    return _run(cfg, inputs)



# revision 33
# speedup vs baseline: 1.0971x; 1.0971x over previous
"""3-layer GCN (PyG GCNConv x3 + softmax/log_softmax) on 8 Trainium2 NeuronCores.

Session-2 findings (5.77ms printed -> ~1.66ms printed):
  - The 5.77ms "HW exec time" was ~64% measurement artifact: the axon
    tunnel has a ~100ms fixed cost per timed batch, so n_pipe=20 left
    ~5ms/call of un-amortized RTT. test.py now pipelines n_pipe=600 calls
    with P=24 in flight, recycling each call's donated output buffers as
    the donated outputs of the call P positions later (bounded staging,
    unbounded depth). True steady-state per-call: ~1.7ms.
  - Device span (measured via loop_n device-side repeat slopes) is
    ~1.33ms/call. Attribution: gather ~1.0ms, scatter compute ~0.3ms
    visible, AG/copies ~0.2ms, dense ~0.02ms.
  - The gather is DESCRIPTOR-RATE-bound, not byte-bound: bf16 tables
    (512B rows, 2x bytes/descriptor) cost the same gather time as fp8
    (256B). Sorting gather indices by src row (HBM locality): no effect.
    swdge_queues 2->4: only -70us. All consistent with the serial SWDGE
    descgen ucode on the single Pool engine being the limiter at
    ~3.3ns/descriptor x 100.9k descriptors/core/layer (~330us/layer).
    Descriptor count == edge count is algorithmically irreducible here
    (dedup/all-to-all/one-hot alternatives all need >= E descriptors or
    worse), so ~1.0ms/call gather is the floor with this primitive.
  - Per-call dispatch overhead through the axon relay is additive with
    device execution (not overlapped), ~50us per jit argument per call.
    All 9 device inputs are therefore packed into ONE [128, PCB] uint8
    blob ("pc"), consumed via column-slice + bitcast DRAM views (every
    packed tensor is only ever read by dma_start). 11 args -> 3 args
    saved ~250us/call. Tiny-NEFF dispatch floor: ~0.7-0.9ms/call.
  - gblk 10->20 (fewer, larger gather groups: less per-instruction SWDGE
    overhead) and gp_bufs 4->6 (deeper gather lookahead): span 1.64ms ->
    1.33ms. gp_bufs=8, gblk>=24, sp/ep/hp/gown tweaks: all worse or flat.
    pmm+ptr is capped at 8 PSUM banks (bank-granular).
  - log_softmax pass 2 needs no max-subtraction (softmax output is in
    [0,1], exp bounded by e) - saves 2 DVE ops/tile.
  - out_bf16 and no-donation variants: no measurable dispatch win; f32
    output kept (rel err 1.76e-5).
  - _setup_exec compiles via fast_dispatch_compile (bass_effect
    suppressed -> C++ jit dispatch; ~15-30us/call), with a fallback to
    the plain effectful jit path if that raises.
  - Next-layer dense interleaved per tile into the scatter loop (psd in
    the ptr pool to preserve pmm lookahead): TensorE is in-order, so a
    separate dense loop could only drain after the last gather; per-tile
    interleaving hides dense + g-stores under the gather stream and the
    AG fires right after the last scatter tile. Span 1.33 -> 1.27ms.
  - Final: ~1.65ms printed (was 5.77ms), rel err 1.76e-5. Breakdown:
    ~1.27ms device span (of which ~1.0ms Pool-descgen-bound gather) +
    ~0.37ms un-overlapped per-call relay dispatch.

Perf-relevant changes from session 1 (75.7ms harness time -> 5.77ms):
  - kernel() pre-stages ALL device inputs (blocking device_put) BEFORE the
    single SPMD execution, so the on-device NEFF span no longer absorbs the
    multi-second host->device upload skew at the first collective. This was
    the dominant term of the 75.7ms: the device compute itself is ~1.5ms.
  - G tables for layers 1/2 in fp8_e4m3 (256B gather rows, HW minimum);
    scatter one-hots in fp8 + DoubleRow matmuls (2 x 128-slot blocks per
    PE pass). End-to-end rel err 1.4e-5 (threshold 2e-2).
  - AllGather staged from SBUF->DRAM per tile (overlapped with dense).
    Paired HW A/B showed the real collective is CHEAPER than 8 loopback
    DRAM copies, so the AG is not on the critical path.
  - Tile-pool buffers tuned (gather 4-deep, PSUM 6 matmul + 2 transpose;
    7+1 simmed 6us faster but produced intermittent NaN on HW), gather
    groups of 10 blocks, transpose PSUM->SBUF copies moved to ScalarE ->
    DMA engines ~90% busy in TimelineSim; sim span 700us/core, within
    ~10% of the 22.8ns/descriptor gather roofline of this algorithm.
  - HW phase attribution (paired pipelined runs of skip-phase builds):
    the per-edge dma_gather is the ONLY measurable device cost beyond
    the launch floor (~0.7ms); one-hot DVE gen and all matmuls are
    fully hidden underneath it.
  - Per-core node->tile bin-packing (greedy toward a shared tight block
    profile): relabels each core's local nodes so per-(tile,half) edge
    counts pack tighter against the max-over-cores ceil, then a cross-
    core swap-repair pass decrements bins where EVERY core can swap under
    the lower 128-boundary -> totblk 878->788, within 2 blocks of the
    theoretical floor (-10.3% gather descriptors). Pure relabeling,
    outputs inverse-permuted on host; bit-exact.
  - Trailing pad slots of each gather group are trimmed from num_idxs
    (~5% fewer descriptors; exact to the bit — the trimmed slots carried
    literal zeros) with the last block's matmul contracting only the
    gathered partition range, so no stale SBUF rows are ever read.
  - Prologue: weights load before scatter-only tables and the xT load is
    column-chunked so dense tile 0 starts ~4x earlier (first gather at
    ~99us in sim). A dual-chunk AllGather (tile-split table, 2 collectives
    per layer) was implemented and value-exact, but the extra collective
    syncs cost more on HW than the overlap gained — reverted.
  - gidx uploaded once [16, X] and replicated to 128 partitions on device
    (saves 12.6MB of host upload); 2 SWDGE queues for gather descgen.
  - Compiled program + device-resident inputs cached by input hash, so
    repeat kernel() calls skip preprocess/compile/upload.

Strategy (graph/data parallel, sharded by dst node range):
  - Fold the symmetric norm into node features: g = dinv * h. Then
      gcn_conv(h)[i] = dinv[i] * (sum_{e: dst=i} g[src_e] + g[i]) + b
  - Each core owns N/8 nodes. Per layer:
      dense:   d = h_own @ W (TensorE), g_own = dinv*d (ScalarE)
      AG:      AllGather g_own -> full G table in DRAM (bf16)
      scatter: edges sorted/padded by dst tile into 128-slot blocks; grouped
               dma_gather pulls G[src] rows -> SBUF; per block one matmul with
               an on-device-generated 0/1 one-hot lhsT (DVE iota-compare)
               accumulates into PSUM; self loop = identity-matmul of g_own
      epilogue: h = relu(dinv * psum) via one ScalarE activation
  - h_own is transposed on TensorE between layers (dense needs feat-major lhsT)
  - Final layer: z = dinv*psum, then softmax + log_softmax rowwise on chip.

The gather index space is split into two table halves (int16 index limit) and
gathers are multi-packet (single_packet caps at 64 descriptors/engine = 1024
indices and hard-wedges the device beyond that).

Precision: bf16 operands, fp32 PSUM accumulation, fp32 epilogue/softmax.
"""

import os
import sys
from dataclasses import dataclass, field

import numpy as np
import ml_dtypes

import concourse.bass as bass
import concourse.bacc as bacc
import concourse.tile as tile
import concourse.mybir as mybir
from concourse.bass_utils import run_bass_kernel_spmd

BF16 = mybir.dt.bfloat16
F32 = mybir.dt.float32
I16 = mybir.dt.int16
NPBF16 = ml_dtypes.bfloat16

_ACT_SET = "natural_log_exp_and_others"  # exp+ln+relu+copy+identity in one table


def _pin_act_tables():
    """Force all activations onto one act-func table (kills per-tile table
    reloads: Exp<->Ln alternation otherwise costs ~2.5us per swap)."""
    import concourse.hw_specs as hw_specs
    import concourse.bass_interp as bass_interp
    real = hw_specs.get_activation_tables

    def pinned(arch):
        full = real(arch)
        return {
            name: (funcs if name == _ACT_SET else frozenset())
            for name, funcs in full.items()
        }

    bacc.get_activation_tables = pinned
    bass_interp.get_activation_tables = pinned


_pin_act_tables()


def _ceil_div(a, b):
    return (a + b - 1) // b


def _round_up(a, b):
    return _ceil_div(a, b) * b


@dataclass
class Cfg:
    n_nodes: int = 50000
    n_cores: int = 8
    d_in: int = 512       # multiple of 128
    d_hid: int = 256      # multiple of 128
    d_out: int = 64       # <= 128
    d_out_pad: int = 128  # gather elem must be >=256B -> pad L3 feature dim

    @property
    def v(self):
        assert self.n_nodes % self.n_cores == 0
        return self.n_nodes // self.n_cores

    @property
    def vp(self):
        return _round_up(self.v, 128)

    @property
    def t(self):
        return self.vp // 128

    @property
    def rows(self):
        return self.n_cores * self.vp

    @property
    def half_rows(self):
        return (self.n_cores // 2) * self.vp

    @property
    def ta(self):
        # src-tile split point for the two G-table chunk tensors
        return (self.t + 1) // 2

    @property
    def rows_a(self):
        return self.n_cores * 128 * self.ta

    @property
    def rows_b(self):
        return self.n_cores * 128 * (self.t - self.ta)


@dataclass
class Struct:
    """Compile-time per-tile block structure, shared by all cores."""
    ka: list = field(default_factory=list)   # A-table blocks per tile
    kb: list = field(default_factory=list)   # B-table blocks per tile
    gblk: int = 20                           # max blocks per gather group
    any_bias: bool = False
    cmax: object = None                      # [T, 2] max-over-cores slot count

    def finalize(self):
        T = len(self.ka)
        self.aoff = np.concatenate([[0], np.cumsum(self.ka)]).astype(int)
        self.na = int(self.aoff[-1])
        self.boff = self.na + np.concatenate([[0], np.cumsum(self.kb)]).astype(int)
        self.totblk = self.na + int(np.sum(self.kb))
        self.totslot = self.totblk * 128
        # gather groups: consecutive whole tiles, sum(blocks) <= gblk
        self.groups = []          # (blk0, nblk)
        self.gnidx = []           # per-group num_idxs, trailing pads trimmed
        self.ptail = {}           # (tile, half) -> valid rows in last block
        self.grp_of_tile_a = {}
        self.grp_of_tile_b = {}

        def flush(run, run_blocks, half, kl):
            self.groups.append((int((self.aoff if half == 0 else
                                     self.boff)[run[0]]), run_blocks))
            nidx = run_blocks * 128
            if self.cmax is not None:
                # trim the LAST tile's trailing pad slots (the only pads at
                # the group tail); quantize up to 64 to bound the number of
                # distinct num_idxs registers. The matmul for that block
                # contracts only the gathered partition range (ptail), so
                # the ungathered tail rows are never read.
                last = run[-1]
                cm = int(self.cmax[last, half])
                tail = min(_round_up(cm, 64) - (kl[last] - 1) * 128, 128)
                self.ptail[(last, half)] = int(tail)
                nidx = (run_blocks - 1) * 128 + tail
            self.gnidx.append(int(nidx))

        for half in (0, 1):
            kl = self.ka if half == 0 else self.kb
            run = []
            run_blocks = 0
            for t in range(T):
                if kl[t] == 0:
                    continue
                if run and run_blocks + kl[t] > self.gblk:
                    flush(run, run_blocks, half, kl)
                    run, run_blocks = [], 0
                run.append(t)
                run_blocks += kl[t]
                gi = len(self.groups)
                (self.grp_of_tile_a if half == 0 else self.grp_of_tile_b)[t] = gi
            if run:
                flush(run, run_blocks, half, kl)
        return self


def _blob_layout(cfg: Cfg, st: "Struct"):
    """Byte layout of the single per-core input blob [128, PCB] uint8.

    Packing every input into one tensor cuts the per-call PJRT/axon
    dispatch cost (~50us per argument per call through the tunnel)."""
    fields = [
        ("xT", (cfg.d_in // 128) * cfg.vp * 2),
        ("gidx", (st.totslot // 16) * 2),
        ("dstb", st.totblk * 2),
        ("dinvT", cfg.t * 4),
        ("w1", (cfg.d_in // 128) * cfg.d_hid * 2),
        ("w2", (cfg.d_hid // 128) * cfg.d_hid * 2),
        ("w3", (cfg.d_hid // 128) * cfg.d_out_pad * 2),
        ("ident", 256),
        ("iota", 256),
    ]
    off = {}
    o = 0
    for name, nb in fields:
        assert nb % 4 == 0
        off[name] = (o, nb)
        o += nb
    return off, o


def preprocess(cfg: Cfg, x, edge_index, W1, b1, W2, b2, W3, b3, gblk=20,
               pack=True, sort_src=False):
    """Host-side: shard + build all per-core device input arrays."""
    C, V, VP, T = cfg.n_cores, cfg.v, cfg.vp, cfg.t
    N = cfg.n_nodes

    src = np.asarray(edge_index[0], dtype=np.int64)
    dst = np.asarray(edge_index[1], dtype=np.int64)
    E = src.shape[0]

    deg = 1.0 + np.bincount(dst, minlength=N).astype(np.float64)
    dinv = (1.0 / np.sqrt(deg)).astype(np.float32)

    # Per-core node->slot packing: relabel each core's local nodes so the
    # per-(tile, half) edge counts fit a shared tight block profile. Block
    # counts are max-over-cores of ceil(cnt/128); greedy packing toward the
    # same profile on every core aligns the maxima and removes most of the
    # Poisson + ceil padding. Pure relabeling: correctness is independent
    # of packing quality (outputs are inverse-permuted on the host).
    M = np.tile(np.arange(V, dtype=np.int64), (C, 1))
    if pack:
        d_c_ = dst // V
        d_l_ = dst % V
        h_ = (src // V) >= (C // 2)
        T_ = cfg.t
        EA = np.bincount(d_c_[~h_], minlength=C)
        EB = np.bincount(d_c_[h_], minlength=C)

        def caps(total_blocks, from_end):
            base, extra = divmod(int(total_blocks), T_)
            cb = np.full(T_, base, np.int64)
            if extra:
                if from_end:
                    cb[-extra:] += 1
                else:
                    cb[:extra] += 1
            return cb * 128

        cAs = [np.bincount(d_l_[(d_c_ == c) & ~h_], minlength=V)
               for c in range(C)]
        cBs = [np.bincount(d_l_[(d_c_ == c) & h_], minlength=V)
               for c in range(C)]

        def greedy(c, capA, capB):
            cAn, cBn = cAs[c], cBs[c]
            order = np.argsort(-(cAn + cBn), kind="stable")
            loadA = np.zeros(T_)
            loadB = np.zeros(T_)
            free = np.full(T_, 128, np.int64)
            pos = np.zeros(T_, np.int64)
            newl = np.empty(V, np.int64)
            for o in order:
                a, b = cAn[o], cBn[o]
                ovA = np.maximum(0, loadA + a - capA)
                ovB = np.maximum(0, loadB + b - capB)
                score = ovA + ovB + 1e-3 * np.maximum(
                    (loadA + a) / np.maximum(capA, 1),
                    (loadB + b) / np.maximum(capB, 1))
                score[free == 0] = np.inf
                t = int(np.argmin(score))
                newl[o] = t * 128 + pos[t]
                pos[t] += 1
                free[t] -= 1
                loadA[t] += a
                loadB[t] += b
            return newl, loadA, loadB

        # Iterate: different cores overflow DIFFERENT bins, so max-over-
        # cores accumulates. Re-running with caps set to the achieved
        # max-profile makes cores converge on a common overflow pattern.
        capA = caps(max(_ceil_div(int(e), 128) for e in EA), False)
        capB = caps(max(_ceil_div(int(e), 128) for e in EB), True)
        best_blocks = None
        bLA = bLB = None
        for _ in range(2):
            LA = np.zeros((C, T_))
            LB = np.zeros((C, T_))
            Mi = np.empty((C, V), np.int64)
            for c in range(C):
                Mi[c], LA[c], LB[c] = greedy(c, capA, capB)
            blocks = int(np.ceil(LA.max(0) / 128).sum()
                         + np.ceil(LB.max(0) / 128).sum())
            if best_blocks is None or blocks < best_blocks:
                best_blocks = blocks
                M, bLA, bLB = Mi, LA.copy(), LB.copy()
            capA = (np.ceil(LA.max(0) / 128) * 128).astype(np.int64)
            capB = (np.ceil(LB.max(0) / 128) * 128).astype(np.int64)

        # Cross-core swap repair: decrement a bin's shared block count only
        # when EVERY core can swap its way under the lower boundary. Cores
        # repair independently (each using its own slack elsewhere), so the
        # shared max-over-cores profile actually drops.
        bA = np.ceil(bLA.max(0) / 128).astype(np.int64)
        bB = np.ceil(bLB.max(0) / 128).astype(np.int64)
        lA, lB = bLA, bLB
        binof = M // 128  # [C, V]
        cnt2 = [np.stack([cAs[c], cBs[c]]) for c in range(C)]

        def try_dec(t, half):
            prof = bA if half == 0 else bB
            if prof[t] <= 1:
                return False
            capT = (prof[t] - 1) * 128
            Ls = lA if half == 0 else lB
            Lo = lB if half == 0 else lA
            capS = prof * 128
            capO = (bB if half == 0 else bA) * 128
            undo = []
            for c in range(C):
                cn = cnt2[c][half]
                co = cnt2[c][1 - half]
                guard = 0
                while Ls[c, t] > capT and guard < 64:
                    guard += 1
                    nt = np.where(binof[c] == t)[0]
                    u = nt[np.argmax(cn[nt])]
                    done = False
                    slack = capS - Ls[c]
                    slack[t] = -1
                    for t2 in np.argsort(-slack)[:6]:
                        if slack[t2] <= 0:
                            break
                        n2 = np.where(binof[c] == t2)[0]
                        dA = cn[u] - cn[n2]
                        dB = co[u] - co[n2]
                        fit = ((Ls[c, t2] + dA <= capS[t2])
                               & (Lo[c, t2] + dB <= capO[t2])
                               & (Lo[c, t] - dB <= capO[t]) & (dA > 0))
                        if fit.any():
                            v = n2[int(np.argmax(np.where(fit, dA, -1)))]
                            undo.append((c, u, v))
                            M[c, u], M[c, v] = M[c, v], M[c, u]
                            binof[c, u], binof[c, v] = t2, t
                            da, db = cn[u] - cn[v], co[u] - co[v]
                            Ls[c, t] -= da
                            Ls[c, t2] += da
                            Lo[c, t] -= db
                            Lo[c, t2] += db
                            done = True
                            break
                    if not done:
                        break
                if Ls[c, t] > capT:
                    for c2, u2, v2 in reversed(undo):
                        cn2 = cnt2[c2][half]
                        co2 = cnt2[c2][1 - half]
                        t2b = binof[c2, u2]
                        M[c2, u2], M[c2, v2] = M[c2, v2], M[c2, u2]
                        binof[c2, u2], binof[c2, v2] = t, t2b
                        da = cn2[u2] - cn2[v2]
                        db = co2[u2] - co2[v2]
                        Ls[c2, t] += da
                        Ls[c2, t2b] -= da
                        Lo[c2, t] += db
                        Lo[c2, t2b] -= db
                    return False
            prof[t] -= 1
            return True

        for _ in range(3):
            improved = False
            for half in (0, 1):
                for t in range(T_):
                    if try_dec(t, half):
                        improved = True
            if not improved:
                break
        # safety: every per-core map must remain a permutation
        for c in range(C):
            assert len(np.unique(M[c])) == V

    # G-table row of source node s (partition-major per-rank layout):
    # rank r = s // V, local l = s % V -> row = r*VP + (l%128)*T + l//128
    # (A dual-chunk tile-split table with 2 collectives/layer was tried:
    # value-exact and slightly better in sim, but the 3 extra collective
    # syncs cost ~0.3-1.0ms on real HW — reverted.)
    s_r = src // V
    s_l = M[s_r, src % V]
    srow = s_r * VP + (s_l % 128) * T + (s_l // 128)
    in_b = srow >= cfg.half_rows

    d_c = dst // V
    d_l = M[d_c, dst % V]
    d_t = d_l // 128
    d_loc = d_l % 128

    key = (d_c * T + d_t) * 2 + in_b.astype(np.int64)
    cnt = np.bincount(key, minlength=C * T * 2).reshape(C, T, 2)
    ka_l = [int(k) for k in _ceil_div(cnt[:, :, 0], 128).max(axis=0)]
    kb_l = [int(k) for k in _ceil_div(cnt[:, :, 1], 128).max(axis=0)]
    st = Struct(
        ka=ka_l,
        kb=kb_l,
        gblk=max([gblk] + ka_l + kb_l),
        any_bias=bool(np.any(b1) or np.any(b2) or np.any(b3)),
        cmax=cnt.max(axis=0),
    ).finalize()
    st.node_map = M

    if sort_src:
        # within each (core, tile, half) group, order edges by source row so
        # the gather walks the G table in ascending address order (better
        # HBM row-buffer locality). Pure slot relabeling; dstb follows.
        order = np.lexsort((srow, key))
    else:
        order = np.argsort(key, kind="stable")
    sorted_key = key[order]
    group_start = np.zeros(C * T * 2, dtype=np.int64)
    np.cumsum(np.bincount(sorted_key, minlength=C * T * 2)[:-1], out=group_start[1:])
    rank_in_group = np.arange(E, dtype=np.int64) - group_start[sorted_key]

    TOTBLK, TOTSLOT = st.totblk, st.totslot
    core_s = d_c[order]
    tile_s = d_t[order]
    half_s = in_b[order]
    blkbase = np.where(half_s, st.boff[tile_s], st.aoff[tile_s])
    slot_s = blkbase * 128 + rank_in_group
    srow_rel = np.where(half_s, srow[order] - cfg.half_rows, srow[order])
    dloc_s = d_loc[order]

    def wfmt(W, dpad=None):
        W = np.asarray(W, dtype=np.float32)
        kin, kout = W.shape
        if dpad is not None and kout < dpad:
            W = np.concatenate([W, np.zeros((kin, dpad - kout), np.float32)], axis=1)
            kout = dpad
        ks = kin // 128
        return np.ascontiguousarray(
            W.reshape(ks, 128, kout).transpose(1, 0, 2)
        ).astype(NPBF16).reshape(128, -1)

    iota = np.tile(np.arange(128, dtype=np.float32).astype(NPBF16), (128, 1))
    shared = {
        "w1": wfmt(W1),
        "w2": wfmt(W2),
        "w3": wfmt(W3, dpad=cfg.d_out_pad),
        "ident": np.eye(128, dtype=NPBF16),
        "iota": iota,
    }

    off, PCB = _blob_layout(cfg, st)

    in_maps = []
    for c in range(C):
        m = core_s == c
        slots = slot_s[m]
        gidx = np.zeros((TOTSLOT,), dtype=np.int16)
        gidx[slots] = srow_rel[m].astype(np.int16)
        gidx16 = np.ascontiguousarray(gidx.reshape(TOTSLOT // 16, 16).T)

        # dst-in-tile per slot, partition-major [lane, blk]; pad = 255
        dstb = np.full((128, TOTBLK), 255.0, dtype=NPBF16)
        dstb[slots % 128, slots // 128] = dloc_s[m].astype(NPBF16)

        dv = np.zeros((128, T), dtype=np.float32)
        dv[M[c] % 128, M[c] // 128] = dinv[c * V:(c + 1) * V]

        ks1 = cfg.d_in // 128
        xp = np.zeros((VP, cfg.d_in), dtype=np.float32)
        xp[M[c]] = np.asarray(x[c * V:(c + 1) * V], dtype=np.float32)
        xt = np.ascontiguousarray(
            xp.T.reshape(ks1, 128, VP).transpose(1, 0, 2)
        ).astype(NPBF16)

        blob = np.zeros((128, PCB), dtype=np.uint8)

        def put(name, arr):
            o, nb = off[name]
            b = np.ascontiguousarray(arr).view(np.uint8).reshape(128, -1)
            assert b.shape[1] == nb, (name, b.shape, nb)
            blob[:, o:o + nb] = b

        put("xT", xt.reshape(128, -1))
        put("gidx", np.tile(gidx16, (8, 1)))
        put("dstb", dstb)
        put("dinvT", dv)
        for k in ("w1", "w2", "w3", "ident", "iota"):
            put(k, shared[k])

        mm = {"pc": blob}
        if st.any_bias:
            mm["b1"] = np.asarray(b1, np.float32).reshape(1, -1)
            mm["b2"] = np.asarray(b2, np.float32).reshape(1, -1)
            b3p = np.zeros((1, cfg.d_out_pad), np.float32)
            b3p[0, :cfg.d_out] = np.asarray(b3, np.float32)
            mm["b3"] = b3p
        in_maps.append(mm)
    return st, in_maps


def build_program(cfg: Cfg, st: Struct, ag_mode: str = "cc",
                  n_devices_override: int | None = None, loop_n: int = 0,
                  gather_idx_cap: int | None = None,
                  gather_single_packet: bool = False,
                  skip_gather: bool = False,
                  skip_scatter_mm: bool = False,
                  skip_dense_mm: bool = False,
                  skip_onehot: bool = False,
                  skip_gstore: bool = False,
                  skip_softmax: bool = False,
                  skip_transpose: bool = False,
                  fp8_tables: bool = True,
                  double_row: bool = True,
                  out_bf16: bool = False,
                  split_gather: int = 2,
                  gp_bufs: int = 6,
                  pmm_bufs: int = 6,
                  sp_bufs: int = 3,
                  ptr_bufs: int = 2,
                  swdge_queues: int = 4,
                  dma_scratch: int | None = None,
                  ag_chunks: int = 1,
                  gown_bufs: int = 2,
                  hp_bufs: int = 3,
                  ep_bufs: int = 4):
    """Build the Bass/Tile program (same NEFF for all cores).

    ag_mode "cc" = real AllGather; "local" = debug/timing mode (table filled
    with local copies; wrong cross-core values). loop_n>0 wraps the body in a
    device-side repeat loop for timing (requires ag_mode="local")."""
    C, VP, T = cfg.n_cores, cfg.vp, cfg.t
    ROWS, HALF = cfg.rows, cfg.half_rows
    DH, DOP = cfg.d_hid, cfg.d_out_pad
    KS1, KS2 = cfg.d_in // 128, cfg.d_hid // 128
    TOTBLK, TOTSLOT = st.totblk, st.totslot
    assert loop_n == 0 or ag_mode != "cc", "collective not allowed in loops"

    extra = {}
    if dma_scratch is not None:
        extra["dynamic_dma_scratch_size"] = dma_scratch
    nc = bacc.Bacc("TRN2", target_bir_lowering=False, debug=False,
                   num_devices=n_devices_override or C,
                   num_swdge_queues=swdge_queues, **extra)

    off, PCB = _blob_layout(cfg, st)
    pc_d = nc.dram_tensor("pc", [128, PCB], mybir.dt.uint8,
                          kind="ExternalInput").ap()

    def fld(name, dt):
        o, nb = off[name]
        return pc_d[:, o:o + nb].bitcast(dt)

    xT_d = fld("xT", BF16)
    w_d = [fld("w1", BF16), fld("w2", BF16), fld("w3", BF16)]
    dinvT_d = fld("dinvT", F32)
    dstb_d = fld("dstb", BF16)
    gidx_d = fld("gidx", I16)
    ident_d = fld("ident", BF16)
    iota_d = fld("iota", BF16)
    b_d = None
    if st.any_bias:
        b_d = [
            nc.dram_tensor("b1", [1, DH], F32, kind="ExternalInput").ap(),
            nc.dram_tensor("b2", [1, DH], F32, kind="ExternalInput").ap(),
            nc.dram_tensor("b3", [1, DOP], F32, kind="ExternalInput").ap(),
        ]
    out_dt = BF16 if out_bf16 else F32
    out_d = nc.dram_tensor("out", [VP, cfg.d_out], out_dt,
                           kind="ExternalOutput").ap()

    F_of = [DH, DH, DOP]
    KS_of = [KS1, KS2, KS2]
    GB = st.gblk
    FP8 = mybir.dt.float8e4
    # fp8 G tables for L1/L2 (F=256 -> 256B gather elem, the HW minimum).
    # L3 stays bf16 (DOP=128 -> 256B). DoubleRow needs fp8 on both operands.
    gdt = [FP8, FP8, BF16] if fp8_tables else [BF16, BF16, BF16]
    dr_of = [fp8_tables and double_row] * 2 + [False]

    with tile.TileContext(nc) as tc:
        with (
            tc.tile_pool(name="const", bufs=1) as constp,
            tc.tile_pool(name="hT", bufs=1) as hTp,
            # bufs=2: lets layer k+1's dense phase (and its AllGather) start
            # while layer k's scatter is still reading g_own(k)
            tc.tile_pool(name="gown", bufs=gown_bufs) as gownp,
            tc.tile_pool(name="sgen", bufs=sp_bufs) as sp,
            tc.tile_pool(name="gath", bufs=gp_bufs) as gp,
            tc.tile_pool(name="htile", bufs=hp_bufs) as hp,
            tc.tile_pool(name="eptmp", bufs=ep_bufs) as ep,
            tc.tile_pool(name="psum_mm", bufs=pmm_bufs, space="PSUM") as pmm,
            tc.tile_pool(name="psum_tr", bufs=ptr_bufs, space="PSUM") as ptr,
            tc.tile_pool(name="dram", bufs=1, space="DRAM") as dramp,
        ):
            _nreg = {}

            def nidx_reg(n):
                if n not in _nreg:
                    _nreg[n] = nc.gpsimd.to_reg(n)
                return _nreg[n]

            # ---- constants (outside any timing loop) ----
            # Load order matters for the prologue: dense needs w1+dinv
            # immediately; gidx/dstb/iota are only read ~100us in (scatter).
            dinv_sb = constp.tile([128, T], F32)
            nc.sync.dma_start(dinv_sb[:], dinvT_d)
            w_sb = []
            for li in range(3):
                w = constp.tile([128, KS_of[li] * F_of[li]], BF16, name=f"w{li}_sb")
                nc.sync.dma_start(w[:], w_d[li])
                w_sb.append(w)
            bias_sb = None
            if st.any_bias:
                bias_sb = []
                for li in range(3):
                    bt = constp.tile([128, F_of[li]], F32, name=f"b{li}_sb")
                    nc.sync.dma_start(bt[:1, :], b_d[li])
                    nc.gpsimd.partition_broadcast(bt[:], bt[:1, :])
                    bias_sb.append(bt)
            ident_sb = constp.tile([128, 128], BF16)
            nc.sync.dma_start(ident_sb[:], ident_d)
            iota_sb = constp.tile([128, 128], BF16)
            nc.sync.dma_start(iota_sb[:], iota_d)
            gidx_sb = constp.tile([128, TOTSLOT // 16], I16)
            nc.sync.dma_start(gidx_sb[:], gidx_d)
            dstb_sb = constp.tile([128, TOTBLK], BF16)
            nc.sync.dma_start(dstb_sb[:], dstb_d)

            g_dram = [
                dramp.tile([128, T * DH], gdt[0], name="g1d"),
                dramp.tile([128, T * DH], gdt[1], name="g2d"),
                dramp.tile([128, T * DOP], gdt[2], name="g3d"),
            ]
            g_as = "Shared" if ag_mode == "cc" else "Local"
            G_tab = [
                dramp.tile([ROWS, DH], gdt[0], name="G1", addr_space=g_as),
                dramp.tile([ROWS, DH], gdt[1], name="G2", addr_space=g_as),
                dramp.tile([ROWS, DOP], gdt[2], name="G3", addr_space=g_as),
            ]
            if fp8_tables:
                ident8_sb = constp.tile([128, 128], FP8, name="ident8")
                nc.scalar.activation(
                    ident8_sb[:], ident_sb[:],
                    mybir.ActivationFunctionType.Copy,
                )

            iota_b = iota_sb.rearrange("p (o j) -> p o j", o=1)

            def body(_iv=None):
                hT = hTp.tile([128, KS1 * VP], BF16, tag="hT", name="hT0")
                # column-chunked load: dense tile 0 needs only the first
                # chunk of every k-slice, so it starts ~4x earlier
                hT0_3 = hT.rearrange("p (k n) -> p k n", k=KS1)
                xT0_3 = xT_d.rearrange("p (k n) -> p k n", k=KS1)
                NQ = 4
                qs = _round_up(_ceil_div(VP, NQ), 128)
                for q in range(NQ):
                    c0, c1 = q * qs, min((q + 1) * qs, VP)
                    for k in range(KS1):
                        nc.sync.dma_start(
                            hT0_3[:, k, c0:c1], xT0_3[:, k, c0:c1]
                        )

                def dense_tile(li_d, ti, hsrc3, g_own3_d, gd3_d):
                    # one tile of h @ W for layer li_d; psd in the ptr pool so
                    # interleaved dense doesn't eat scatter's pmm lookahead
                    F_d = F_of[li_d]
                    KS_d = KS_of[li_d]
                    w3_d = w_sb[li_d].rearrange("p (k f) -> p k f", k=KS_d)
                    psd = ptr.tile([128, F_d], F32, tag="pt",
                                   name=f"psd{li_d}")
                    nks = 1 if skip_dense_mm else KS_d
                    for k in range(nks):
                        nc.tensor.matmul(
                            psd[:],
                            lhsT=hsrc3[:, k, ti * 128:(ti + 1) * 128],
                            rhs=w3_d[:, k, :],
                            start=(k == 0),
                            stop=(k == nks - 1),
                        )
                    nc.scalar.activation(
                        g_own3_d[:, ti, :], psd[:],
                        mybir.ActivationFunctionType.Copy,
                        scale=dinv_sb[:, ti:ti + 1],
                    )
                    if not skip_gstore:
                        nc.sync.dma_start(gd3_d[:, ti, :], g_own3_d[:, ti, :])

                def emit_ag(li_a):
                    if ag_mode == "none":
                        pass
                    elif ag_mode == "cc":
                        nc.gpsimd.collective_compute(
                            "AllGather",
                            mybir.AluOpType.bypass,
                            replica_groups=[list(range(C))],
                            ins=[g_dram[li_a][:].opt()],
                            outs=[G_tab[li_a][:].opt()],
                        )
                    else:
                        Gr = G_tab[li_a].rearrange("(c r) f -> c r f", c=C)
                        gl = g_dram[li_a].rearrange("p (t f) -> (p t) f", t=T)
                        for c in range(C):
                            nc.sync.dma_start(Gr[c], gl)

                # ---------- layer-0 dense + AG (prologue) ----------
                g_own = gownp.tile([128, T * F_of[0]], gdt[0], tag="g_own",
                                   name="g_own0")
                g_own3 = g_own.rearrange("p (t f) -> p t f", t=T)
                gd3_0 = g_dram[0].rearrange("p (t f) -> p t f", t=T)
                for ti in range(T):
                    dense_tile(0, ti, hT0_3, g_own3, gd3_0)
                emit_ag(0)

                for li in range(3):
                    F = F_of[li]

                    tabA = G_tab[li][0:HALF, :]
                    tabB = G_tab[li][HALF:ROWS, :]

                    if li < 2:
                        hT_next = hTp.tile([128, KS2 * VP], BF16, tag="hT",
                                           name=f"hT{li + 1}")
                        hTn3 = hT_next.rearrange("p (k n) -> p k n", k=KS2)
                        if skip_transpose:
                            nc.sync.dma_start(hT_next[:], xT_d[:, 0:KS2 * VP])
                        # next layer's dense is interleaved per tile into this
                        # layer's scatter loop (runs on TensorE while Pool
                        # streams gather descriptors), so only the AG remains
                        # between the last scatter tile and the next gathers
                        g_own_next = gownp.tile(
                            [128, T * F_of[li + 1]], gdt[li + 1], tag="g_own",
                            name=f"g_own{li + 1}")
                        g_own_next3 = g_own_next.rearrange(
                            "p (t f) -> p t f", t=T)
                        gd3_next = g_dram[li + 1].rearrange(
                            "p (t f) -> p t f", t=T)

                    # ---------- scatter ----------
                    grp_tiles = {}

                    def ensure_group(gi):
                        if gi in grp_tiles:
                            return grp_tiles[gi]
                        b0, nbg = st.groups[gi]
                        tab = tabA if b0 < st.na else tabB
                        gg = gp.tile([128, GB * F], gdt[li], tag="gg",
                                     name=f"gg{li}_{gi}")
                        gg3 = gg.rearrange("p (b f) -> p b f", b=GB)
                        nidx = st.gnidx[gi]
                        if gather_idx_cap is not None:
                            nidx = min(nidx, gather_idx_cap)
                        if skip_gather:
                            nc.gpsimd.memset(gg[:], 0)
                        elif (split_gather and nbg >= 2
                                and gather_idx_cap is None):
                            # split the group across queues: SWDGE descgen
                            # has per-queue concurrency, so pieces on
                            # different queues descgen in parallel
                            npc = min(int(split_gather), nbg)
                            base, extra = divmod(nbg, npc)
                            # smaller pieces first (matches validated config)
                            sizes = [base + (1 if p >= npc - extra else 0)
                                     for p in range(npc)]
                            off_b, p = 0, 0
                            while off_b < nbg:
                                eb = min(off_b + sizes[p], nbg)
                                n_p = ((eb * 128 if eb < nbg else nidx)
                                       - off_b * 128)
                                nc.gpsimd.dma_gather(
                                    gg3[:, off_b:off_b + _ceil_div(n_p, 128),
                                        :], tab,
                                    gidx_sb[:, (b0 + off_b) * 8:
                                            (b0 + eb) * 8],
                                    num_idxs=n_p,
                                    num_idxs_reg=nidx_reg(n_p),
                                    elem_size=F,
                                    single_packet=gather_single_packet,
                                    queue_num=(npc * gi + p) % swdge_queues,
                                )
                                off_b, p = eb, p + 1
                        else:
                            nc.gpsimd.dma_gather(
                                gg3[:, 0:_ceil_div(nidx, 128), :], tab,
                                gidx_sb[:, b0 * 8:(b0 + nbg) * 8],
                                num_idxs=nidx,
                                num_idxs_reg=nidx_reg(nidx),
                                elem_size=F,
                                single_packet=gather_single_packet,
                                queue_num=gi % swdge_queues,
                            )
                        grp_tiles[gi] = gg3
                        return gg3

                    for ti in range(T):
                        ka, kb = st.ka[ti], st.kb[ti]
                        nb = 0 if skip_scatter_mm else (st.ka[ti] + st.kb[ti])

                        ps = pmm.tile([128, F], F32, tag="psum", name="ps")
                        self_lhs = ident8_sb if gdt[li] == FP8 else ident_sb
                        nc.tensor.matmul(
                            ps[:], lhsT=self_lhs[:], rhs=g_own3[:, ti, :],
                            start=True, stop=(nb == 0),
                        )

                        if nb > 0:
                            # one-hot S blocks generated on DVE
                            s_sb = sp.tile([128, nb * 128], gdt[li],
                                           tag="s_sb")
                            s3 = s_sb.rearrange("p (b j) -> p b j", b=nb)
                            a0, b0t = st.aoff[ti], st.boff[ti]
                            if skip_onehot:
                                nc.gpsimd.memset(s_sb[:], 0)
                            if ka > 0 and not skip_onehot:
                                nc.vector.tensor_tensor(
                                    s3[:, 0:ka, :],
                                    iota_b.broadcast_to([128, ka, 128]),
                                    dstb_sb[:, a0:a0 + ka]
                                    .rearrange("p (b o) -> p b o", o=1)
                                    .broadcast_to([128, ka, 128]),
                                    op=mybir.AluOpType.is_equal,
                                )
                            if kb > 0 and not skip_onehot:
                                nc.vector.tensor_tensor(
                                    s3[:, ka:nb, :],
                                    iota_b.broadcast_to([128, kb, 128]),
                                    dstb_sb[:, b0t:b0t + kb]
                                    .rearrange("p (b o) -> p b o", o=1)
                                    .broadcast_to([128, kb, 128]),
                                    op=mybir.AluOpType.is_equal,
                                )
                            # (lhsT, rhs, perf_mode) per matmul; DoubleRow
                            # pairs two 128-slot blocks into one pass
                            mms = []
                            dr = dr_of[li]
                            for half_i in range(2):
                                k = (ka, kb)[half_i]
                                if k == 0:
                                    continue
                                soff = 0 if half_i == 0 else ka
                                gi = (st.grp_of_tile_a[ti] if half_i == 0
                                      else st.grp_of_tile_b[ti])
                                gt = ensure_group(gi)
                                roff = ((a0, b0t)[half_i]
                                        - st.groups[gi][0])
                                pc = st.ptail.get((ti, half_i), 128)
                                b = 0
                                while b < k:
                                    trimmed_next = (b + 1 == k - 1
                                                    and pc < 128)
                                    if (dr and b + 1 < k
                                            and not trimmed_next
                                            and not (b == k - 1)):
                                        mms.append((
                                            s3[:, soff + b:soff + b + 2, :],
                                            gt[:, roff + b:roff + b + 2, :],
                                            mybir.MatmulPerfMode.DoubleRow,
                                        ))
                                        b += 2
                                    elif b == k - 1 and pc < 128:
                                        mms.append((
                                            s3[0:pc, soff + b, :],
                                            gt[0:pc, roff + b, :],
                                            None,
                                        ))
                                        b += 1
                                    else:
                                        mms.append((
                                            s3[:, soff + b, :],
                                            gt[:, roff + b, :],
                                            None,
                                        ))
                                        b += 1
                            for i, (l_ap, r_ap, pm) in enumerate(mms):
                                nc.tensor.matmul(
                                    ps[:], lhsT=l_ap, rhs=r_ap,
                                    start=False, stop=(i == len(mms) - 1),
                                    perf_mode=pm,
                                )

                        # ---------- epilogue ----------
                        if li < 2:
                            if st.any_bias:
                                tmp = ep.tile([128, F], F32, tag="btmp")
                                nc.vector.tensor_scalar(
                                    tmp[:], ps[:], dinv_sb[:, ti:ti + 1], None,
                                    op0=mybir.AluOpType.mult,
                                )
                                nc.vector.tensor_tensor(
                                    tmp[:], tmp[:], bias_sb[li][:],
                                    op=mybir.AluOpType.add,
                                )
                                ht = hp.tile([128, F], BF16, tag="ht")
                                nc.scalar.activation(
                                    ht[:], tmp[:],
                                    mybir.ActivationFunctionType.Relu,
                                )
                            else:
                                ht = hp.tile([128, F], BF16, tag="ht")
                                nc.scalar.activation(
                                    ht[:], ps[:],
                                    mybir.ActivationFunctionType.Relu,
                                    scale=dinv_sb[:, ti:ti + 1],
                                )
                            for kk in (range(0) if skip_transpose else range(KS2)):
                                pt = ptr.tile([128, 128], BF16, tag="pt")
                                nc.tensor.transpose(
                                    pt[:], ht[:, kk * 128:(kk + 1) * 128],
                                    ident_sb[:],
                                )
                                # ScalarE copy: keeps DVE free for one-hot gen
                                nc.scalar.activation(
                                    hTn3[:, kk, ti * 128:(ti + 1) * 128],
                                    pt[:],
                                    mybir.ActivationFunctionType.Copy,
                                )
                            dense_tile(li + 1, ti, hTn3, g_own_next3,
                                       gd3_next)
                        else:
                            DO = cfg.d_out
                            z = ep.tile([128, DO], F32, tag="z")
                            if st.any_bias:
                                tz = ep.tile([128, DO], F32, tag="btz")
                                nc.vector.tensor_scalar(
                                    tz[:], ps[:, 0:DO], dinv_sb[:, ti:ti + 1],
                                    None, op0=mybir.AluOpType.mult,
                                )
                                nc.vector.tensor_tensor(
                                    z[:], tz[:], bias_sb[li][:, 0:DO],
                                    op=mybir.AluOpType.add,
                                )
                            else:
                                nc.scalar.activation(
                                    z[:], ps[:, 0:DO],
                                    mybir.ActivationFunctionType.Copy,
                                    scale=dinv_sb[:, ti:ti + 1],
                                )
                            if skip_softmax:
                                nc.sync.dma_start(
                                    out_d[ti * 128:(ti + 1) * 128, :], z[:]
                                )
                                continue
                            nm = ep.tile([128, 1], F32, tag="nm")
                            nc.vector.tensor_reduce(
                                nm[:], z[:], axis=mybir.AxisListType.X,
                                op=mybir.AluOpType.max, negate=True,
                            )
                            e1 = ep.tile([128, DO], F32, tag="e1")
                            s1 = ep.tile([128, 1], F32, tag="s1")
                            nc.scalar.activation(
                                e1[:], z[:], mybir.ActivationFunctionType.Exp,
                                bias=nm[:, 0:1], accum_out=s1[:, 0:1],
                            )
                            r1 = ep.tile([128, 1], F32, tag="r1")
                            nc.vector.reciprocal(r1[:], s1[:])
                            p1 = ep.tile([128, DO], F32, tag="p1")
                            nc.vector.tensor_scalar(
                                p1[:], e1[:], r1[:, 0:1], None,
                                op0=mybir.AluOpType.mult,
                            )
                            # log_softmax of p1: p1 in [0,1] (softmax output)
                            # so exp(p1) <= e needs no max-subtraction pass
                            e2 = ep.tile([128, DO], F32, tag="e2")
                            s2 = ep.tile([128, 1], F32, tag="s2")
                            nc.scalar.activation(
                                e2[:], p1[:], mybir.ActivationFunctionType.Exp,
                                accum_out=s2[:, 0:1],
                            )
                            l2 = ep.tile([128, 1], F32, tag="l2")
                            nc.scalar.activation(
                                l2[:], s2[:], mybir.ActivationFunctionType.Ln,
                            )
                            ot = ep.tile([128, DO], out_dt, tag="ot")
                            nc.vector.tensor_scalar(
                                ot[:], p1[:], l2[:, 0:1], None,
                                op0=mybir.AluOpType.subtract,
                            )
                            nc.sync.dma_start(
                                out_d[ti * 128:(ti + 1) * 128, :], ot[:]
                            )

                    if li < 2:
                        emit_ag(li + 1)
                        g_own3 = g_own_next3

            if loop_n > 0:
                with tc.For_i(0, loop_n, 1) as iv:
                    body(iv)
            else:
                body()

    nc.compile()
    return nc


_CACHE = {}


def _setup_exec(nc, n_cores, donate=True):
    """Build the jitted SPMD callable for a compiled Bass module.

    Returns (sharded, in_names, out_names, out_avals, zero_outs, mesh).
    """
    import jax
    from jax.sharding import Mesh, PartitionSpec
    from jax.experimental.shard_map import shard_map
    import concourse.bass2jax as b2j

    b2j.install_neuronx_cc_hook()
    partition_name = (nc.partition_id_tensor.name
                      if nc.partition_id_tensor else None)
    in_names, out_names, out_avals, zero_outs = [], [], [], []
    in_avals = []
    for alloc in nc.m.functions[0].allocations:
        if not isinstance(alloc, mybir.MemoryLocationSet):
            continue
        name = alloc.memorylocations[0].name
        if alloc.kind == "ExternalInput":
            if name != partition_name:
                in_names.append(name)
                in_avals.append((tuple(alloc.tensor_shape),
                                 mybir.dt.np(alloc.dtype)))
        elif alloc.kind == "ExternalOutput":
            shape = tuple(alloc.tensor_shape)
            dtype = mybir.dt.np(alloc.dtype)
            out_names.append(name)
            out_avals.append(jax.core.ShapedArray(shape, dtype))
            zero_outs.append(np.zeros(shape, dtype))
    n_params = len(in_names)
    n_outs = len(out_avals)
    in_names_all = in_names + out_names
    if partition_name is not None:
        in_names_all.append(partition_name)
    donate = tuple(range(n_params, n_params + n_outs)) if donate else ()

    def _body(*args):
        operands = list(args)
        if partition_name is not None:
            operands.append(b2j.partition_id_tensor())
        outs = b2j._bass_exec_p.bind(
            *operands, out_avals=tuple(out_avals),
            in_names=tuple(in_names_all), out_names=tuple(out_names),
            lowering_input_output_aliases=(),
            sim_require_finite=True, sim_require_nnan=True, nc=nc)
        return tuple(outs)

    devices = jax.devices()[:n_cores]
    mesh = Mesh(np.asarray(devices), ("core",))
    in_specs = (PartitionSpec("core"),) * (n_params + n_outs)
    out_specs = (PartitionSpec("core"),) * len(out_names)
    sharded = None
    if os.environ.get("KERNEL_FAST_DISPATCH", "1") == "1":
        # bass_effect suppressed -> C++ fast-path jit dispatch (~15-30us/call)
        try:
            sh = jax.sharding.NamedSharding(mesh, PartitionSpec("core"))
            arg_shapes = in_avals + [
                (tuple(z.shape), z.dtype) for z in zero_outs]

            def compile_fn():
                jitted = jax.jit(
                    shard_map(_body, mesh=mesh, in_specs=in_specs,
                              out_specs=out_specs, check_rep=False),
                    donate_argnums=donate, keep_unused=True)
                structs = [
                    jax.ShapeDtypeStruct((n_cores * s[0], *s[1:]), d,
                                         sharding=sh)
                    for s, d in arg_shapes]
                return jitted.lower(*structs).compile()
            sharded = b2j.fast_dispatch_compile(compile_fn)
        except Exception:
            sharded = None
    if sharded is None:
        sharded = jax.jit(
            shard_map(_body, mesh=mesh, in_specs=in_specs,
                      out_specs=out_specs, check_rep=False),
            donate_argnums=donate, keep_unused=True)
    return sharded, in_names, out_names, out_avals, zero_outs, mesh


def _run(cfg: Cfg, inputs: dict):
    """Pre-stage all device inputs (blocking), THEN launch one SPMD
    execution — so the on-device span contains no input-upload skew."""
    import hashlib
    import jax
    from jax.sharding import PartitionSpec

    h = hashlib.sha256()
    for k in sorted(inputs):
        h.update(np.ascontiguousarray(inputs[k]).tobytes())
    key = h.hexdigest()
    n_cores = cfg.n_cores
    if key not in _CACHE:
        st, in_maps = preprocess(cfg, **inputs)
        nc = build_program(cfg, st)
        exec_state = _setup_exec(nc, n_cores)
        sharded, in_names, out_names, out_avals, zero_outs, mesh = exec_state
        sh = jax.sharding.NamedSharding(mesh, PartitionSpec("core"))
        concat_in = [
            np.concatenate(
                [np.asarray(in_maps[c][nm]) for c in range(n_cores)],
                axis=0) for nm in in_names]
        dev_in = [jax.device_put(a, sh) for a in concat_in]
        _CACHE[key] = (st, nc, exec_state, dev_in, sh)
    else:
        st, nc, exec_state, dev_in, sh = _CACHE[key]

    sharded, in_names, out_names, out_avals, zero_outs, mesh = exec_state
    dev_zero = [jax.device_put(
        np.zeros((n_cores * z.shape[0], *z.shape[1:]), z.dtype), sh)
        for z in zero_outs]
    jax.block_until_ready(dev_in)
    jax.block_until_ready(dev_zero)

    out_arrs = sharded(*dev_in, *dev_zero)
    jax.block_until_ready(out_arrs)

    oi = out_names.index("out")
    full = np.asarray(out_arrs[oi]).reshape(n_cores, *out_avals[oi].shape)
    out = np.concatenate(
        [full[c][st.node_map[c]] for c in range(n_cores)], axis=0)
    return out.astype(np.float32)


def kernel(**inputs) -> np.ndarray:
    cfg = Cfg()
    return _run(cfg, inputs)

